# revision 2
# baseline (speedup 1.0000x reference)
"""DepthWeightedAssignment Trainium2 kernel.

Per-row (detection) argmin over 64 cameras of
  cost[i,j] = (d_i-c_j)^2 + 0.5*(1-exp(-0.045 c_j)) + 0.3*(t_i-t_j)^2/3600
plus threshold/weight postprocessing, sharded over 8 NeuronCores (N axis).

Device algorithm (per core, N_c = 131072 detections):
  - PE matmul computes V = fl(49152*cost + 2^30) with the 2^30 row
    accumulated last, so V - 2^30 = 64*k exactly, k = round(768*cost).
    Layout: PSUM [128 partitions = (A cams0-31 | A cams32-63 | B cams0-31 |
    B cams32-63), free = detection columns]; two detections (A,B) share each
    moving column via disjoint K-rows.
  - ACT subtracts 2^30 (exact power-of-two bias) -> Y = 64*k.
  - GPSIMD adds per-partition camera index j -> X = 64*k + j (exact fp32
    integers for any row-minimum; losers stay strictly larger).
  - DVE tensor_reduce(min, axis=X, apply_transpose=True) reduces each
    32-camera partition group into the free dim (32x32 stream transpose),
    all 128 lanes active.
  - Small TT-min combines the two 32-cam groups; int unpack j = X & 63,
    k = X >> 6; weights = valid/(1+sqrt(k/768)).
Host side only shards inputs, builds O(64) camera coefficient tables, and
un-permutes the outputs.
"""
import sys

sys.path.insert(0, "/opt/trn_rl_repo")

import numpy as np

N_TOTAL = 1 << 20
M_CAMS = 64
N_CORES = 8
N_C = N_TOTAL // N_CORES          # 131072 per core
HALF = N_C // 2                   # 65536 (A half / B half)
CH = 8192                         # moving columns per M-tile
NQ = HALF // CH                   # 8 M-tiles per core
REG = 2048                        # psum region columns
NREG = CH // REG                  # 4 regions per M-tile
S_FINE = 768.0
SS = 64.0 * S_FINE                # 49152
H_ROW = 2.0 ** 30
W_T = 0.3 / 3600.0
DC = 100.0                        # depth centering
TCEN = 1800.0                     # time centering
THRESH_K = 625.0 * S_FINE         # 480000

_CACHE = {}


def _build_module():
    import concourse.bacc as bacc
    import concourse.tile as tile
    from concourse import mybir

    f32 = mybir.dt.float32
    i32 = mybir.dt.int32
    AF = mybir.ActivationFunctionType
    OP = mybir.AluOpType
    AX = mybir.AxisListType

    nc = bacc.Bacc("TRN2", target_bir_lowering=False)

    ddep = nc.dram_tensor("ddep", [N_C], f32, kind="ExternalInput")
    dtim = nc.dram_tensor("dtim", [N_C], f32, kind="ExternalInput")
    stat_in = nc.dram_tensor("stat", [10, 128], f32, kind="ExternalInput")
    jb_in = nc.dram_tensor("jbias", [128, 1], f32, kind="ExternalInput")
    ones_in = nc.dram_tensor("onesrow", [1, CH], f32, kind="ExternalInput")
    asn_out = nc.dram_tensor("asn", [64, 2 * CH // 8], i32, kind="ExternalOutput")
    wts_out = nc.dram_tensor("wts", [64, 2 * CH // 8], f32, kind="ExternalOutput")
    # free width = NQ*CH/32 = 8*256 = 2048 per partition
    assert asn_out.shape == [64, 2048]

    with tile.TileContext(nc) as tc:
        with (
            tc.tile_pool(name="const", bufs=1) as cpool,
            tc.tile_pool(name="prep", bufs=1) as prep,
            tc.tile_pool(name="load", bufs=2) as load,
            tc.tile_pool(name="mov", bufs=2) as mpool,
            tc.tile_pool(name="ps", bufs=2, space="PSUM") as ppool,
            tc.tile_pool(name="y", bufs=2) as ypool,
            tc.tile_pool(name="x", bufs=2) as xpool,
            tc.tile_pool(name="s1", bufs=2) as s1pool,
            tc.tile_pool(name="sh", bufs=2) as shpool,
            tc.tile_pool(name="fin", bufs=1) as fpool,
            tc.tile_pool(name="post", bufs=1) as qpool,
        ):
            # ---- constants ----
            stat_t = cpool.tile([10, 128], f32)
            nc.sync.dma_start(stat_t[:], stat_in[:])
            jb = cpool.tile([128, 1], f32)
            nc.sync.dma_start(jb[:], jb_in[:])
            bias_h = cpool.tile([128, 1], f32)
            nc.gpsimd.memset(bias_h[:], -(2.0 ** 30))
            bias_d = cpool.tile([128, 1], f32)
            nc.gpsimd.memset(bias_d[:], -DC)
            bias_t = cpool.tile([128, 1], f32)
            nc.gpsimd.memset(bias_t[:], -TCEN)

            # ---- prep: centered values and squares, [128, 512] per half ----
            # layout: half h tile[p, f] = det h*65536 + p*512 + f
            prep_tiles = {}
            for h in ("A", "B"):
                off = 0 if h == "A" else HALF
                dload = load.tile([128, 512], f32, tag="dload")
                nc.sync.dma_start(
                    dload[:], ddep[off:off + HALF].rearrange("(p f) -> p f", f=512)
                )
                d1 = prep.tile([128, 512], f32, tag=f"d1{h}")
                nc.scalar.activation(d1[:], dload[:], AF.Identity, bias=bias_d[:])
                d2 = prep.tile([128, 512], f32, tag=f"d2{h}")
                nc.scalar.activation(d2[:], dload[:], AF.Square, bias=bias_d[:])
                tload = load.tile([128, 512], f32, tag="tload")
                nc.sync.dma_start(
                    tload[:], dtim[off:off + HALF].rearrange("(p f) -> p f", f=512)
                )
                t1 = prep.tile([128, 512], f32, tag=f"t1{h}")
                nc.scalar.activation(t1[:], tload[:], AF.Identity, bias=bias_t[:])
                t2 = prep.tile([128, 512], f32, tag=f"t2{h}")
                nc.scalar.activation(t2[:], tload[:], AF.Square, bias=bias_t[:])
                prep_tiles[h] = (d1, d2, t1, t2)

            fa = fpool.tile([64, NQ * 256], f32)

            # ---- main loop over M-tiles ----
            for q in range(NQ):
                m = mpool.tile([10, CH], f32, tag="m")
                nc.sync.dma_start(m[0:1, :], ones_in[:])
                nc.sync.dma_start(m[9:10, :], ones_in[:])
                rows = [
                    prep_tiles["A"][0], prep_tiles["A"][1],
                    prep_tiles["A"][2], prep_tiles["A"][3],
                    prep_tiles["B"][0], prep_tiles["B"][1],
                    prep_tiles["B"][2], prep_tiles["B"][3],
                ]
                for ri, srct in enumerate(rows):
                    nc.sync.dma_start(
                        m[1 + ri:2 + ri, :].rearrange("o (p f) -> o p f", p=16),
                        srct[16 * q:16 * q + 16, :],
                    )

                s1 = s1pool.tile([128, NREG * 64], f32, tag="s1")
                for r in range(NREG):
                    ps = ppool.tile([128, REG], f32, tag="ps")
                    for c in range(REG // 512):
                        col = r * REG + c * 512
                        nc.tensor.matmul(
                            ps[:, c * 512:(c + 1) * 512],
                            stat_t[:],
                            m[:, col:col + 512],
                            start=True,
                            stop=True,
                        )
                    y = ypool.tile([128, REG], f32, tag="y")
                    nc.scalar.activation(y[:], ps[:], AF.Identity, bias=bias_h[:])
                    x = xpool.tile([128, REG], f32, tag="x")
                    nc.gpsimd.tensor_scalar(
                        out=x[:], in0=y[:], scalar1=jb[:], scalar2=None, op0=OP.add
                    )
                    nc.vector.tensor_reduce(
                        out=s1[:, r * 64:(r + 1) * 64],
                        in_=x[:].rearrange("p (b j) -> p b j", j=32),
                        op=OP.min,
                        axis=AX.X,
                        apply_transpose=True,
                    )

                # stage 2: combine 32-cam groups (partition shift via DMA)
                shg1 = shpool.tile([64, 256], f32, tag="shg1")
                nc.sync.dma_start(shg1[0:32, :], s1[32:64, :])
                nc.sync.dma_start(shg1[32:64, :], s1[96:128, :])
                shb0 = shpool.tile([64, 256], f32, tag="shb0")
                nc.sync.dma_start(shb0[32:64, :], s1[64:96, :])
                nc.vector.tensor_tensor(
                    out=fa[0:32, q * 256:(q + 1) * 256],
                    in0=s1[0:32, :], in1=shg1[0:32, :], op=OP.min,
                )
                nc.vector.tensor_tensor(
                    out=fa[32:64, q * 256:(q + 1) * 256],
                    in0=shb0[32:64, :], in1=shg1[32:64, :], op=OP.min,
                )

            # ---- postprocess in two column chunks of 1024 ----
            for ci in range(2):
                cs = slice(ci * 1024, (ci + 1) * 1024)
                ui = qpool.tile([64, 1024], i32, tag="ui")
                nc.vector.tensor_copy(ui[:], fa[:, cs])
                ji = qpool.tile([64, 1024], i32, tag="ji")
                nc.vector.tensor_scalar(
                    out=ji[:], in0=ui[:], scalar1=63, scalar2=None, op0=OP.bitwise_and
                )
                nc.vector.tensor_scalar(
                    out=ui[:], in0=ui[:], scalar1=6, scalar2=None,
                    op0=OP.arith_shift_right,
                )
                kf = qpool.tile([64, 1024], f32, tag="kf")
                nc.vector.tensor_copy(kf[:], ui[:])
                jf = qpool.tile([64, 1024], f32, tag="jf")
                nc.vector.tensor_copy(jf[:], ji[:])
                valid = qpool.tile([64, 1024], f32, tag="valid")
                nc.vector.tensor_scalar(
                    out=valid[:], in0=kf[:], scalar1=THRESH_K, scalar2=None,
                    op0=OP.is_lt,
                )
                # assignments = (jf + 1) * valid - 1
                nc.vector.tensor_scalar(
                    out=jf[:], in0=jf[:], scalar1=1.0, scalar2=None, op0=OP.add
                )
                nc.vector.tensor_tensor(out=jf[:], in0=jf[:], in1=valid[:], op=OP.mult)
                nc.vector.tensor_scalar(
                    out=jf[:], in0=jf[:], scalar1=1.0, scalar2=None, op0=OP.subtract
                )
                nc.vector.tensor_copy(ji[:], jf[:])
                nc.sync.dma_start(asn_out[:, cs], ji[:])
                # weights = valid / (1 + sqrt(kf/768))
                nc.vector.tensor_scalar(
                    out=kf[:], in0=kf[:], scalar1=1.0 / S_FINE, scalar2=None,
                    op0=OP.mult,
                )
                sq = qpool.tile([64, 1024], f32, tag="sq")
                nc.scalar.activation(sq[:], kf[:], AF.Sqrt)
                nc.vector.tensor_scalar(
                    out=sq[:], in0=sq[:], scalar1=1.0, scalar2=None, op0=OP.add
                )
                nc.vector.reciprocal(sq[:], sq[:])
                nc.vector.tensor_tensor(out=sq[:], in0=sq[:], in1=valid[:], op=OP.mult)
                nc.sync.dma_start(wts_out[:, cs], sq[:])

    nc.compile()
    return nc


def _host_consts(camera_depths, camera_times):
    cd = np.asarray(camera_depths, np.float64)
    ct = np.asarray(camera_times, np.float64)
    c1 = cd - DC
    t1 = ct - TCEN
    L = 0.5 * (1.0 - np.exp(-0.045 * cd))
    a = SS * (c1 * c1 + W_T * t1 * t1 + L)      # ones-row coeff
    b_d = SS * (-2.0 * c1)                      # d' coeff
    b_t = SS * (-2.0 * W_T * t1)                # t' coeff
    stat = np.zeros((10, 128), np.float32)
    jb = np.zeros((128, 1), np.float32)
    for g in range(4):
        cams = np.arange(32) + 32 * (g % 2)
        cols = slice(32 * g, 32 * g + 32)
        stat[0, cols] = a[cams]
        if g < 2:
            stat[1, cols] = b_d[cams]
            stat[2, cols] = SS
            stat[3, cols] = b_t[cams]
            stat[4, cols] = SS * W_T
        else:
            stat[5, cols] = b_d[cams]
            stat[6, cols] = SS
            stat[7, cols] = b_t[cams]
            stat[8, cols] = SS * W_T
        jb[cols, 0] = cams
    stat[9, :] = H_ROW
    ones = np.ones((1, CH), np.float32)
    return stat, jb, ones


def _det_perm():
    """device (p, m) -> core-local detection index, flattened [64*2048]."""
    p = np.arange(64)[:, None]
    m = np.arange(2048)[None, :]
    half = p // 32
    i = p % 32
    q = m >> 8
    r = (m >> 6) & 3
    b = m & 63
    n = 2048 * r + 32 * b + i
    det = half * HALF + CH * q + 512 * (n >> 9) + (n & 511)
    return det.ravel()


def kernel(detection_depths, camera_depths, detection_times, camera_times):
    from concourse.bass_utils import run_bass_kernel_spmd

    if "nc" not in _CACHE:
        _CACHE["nc"] = _build_module()
        _CACHE["perm"] = _det_perm()
    nc = _CACHE["nc"]
    perm = _CACHE["perm"]

    dd = np.ascontiguousarray(np.asarray(detection_depths, np.float32))
    dt = np.ascontiguousarray(np.asarray(detection_times, np.float32))
    stat, jb, ones = _host_consts(camera_depths, camera_times)

    in_maps = []
    for c in range(N_CORES):
        sl = slice(c * N_C, (c + 1) * N_C)
        in_maps.append({
            "ddep": dd[sl].copy(),
            "dtim": dt[sl].copy(),
            "stat": stat,
            "jbias": jb,
            "onesrow": ones,
        })
    results = run_bass_kernel_spmd(nc, in_maps, list(range(N_CORES))).results

    assignments = np.empty(N_TOTAL, np.int32)
    weights = np.empty(N_TOTAL, np.float32)
    for c in range(N_CORES):
        base = c * N_C
        a_loc = np.empty(N_C, np.int32)
        w_loc = np.empty(N_C, np.float32)
        a_loc[perm] = results[c]["asn"].ravel()
        w_loc[perm] = results[c]["wts"].ravel()
        assignments[base:base + N_C] = a_loc
        weights[base:base + N_C] = w_loc
    return assignments, weights


# revision 16
# speedup vs baseline: 711.0276x; 711.0276x over previous
"""DepthWeightedAssignment Trainium2 kernel.

Per-row (detection) argmin over 64 cameras of
  cost[i,j] = (d_i-c_j)^2 + 0.5*(1-exp(-0.045 c_j)) + 0.3*(t_i-t_j)^2/3600
plus threshold/weight postprocessing, sharded over 8 NeuronCores (N axis).

Device algorithm (per core, N_c = 131072 detections):
  - PE matmul computes V = fl(49152*cost + 2^30) with the 2^30 row
    accumulated last, so V - 2^30 = 64*k exactly, k = round(768*cost).
    Layout: PSUM [128 partitions = (A cams0-31 | A cams32-63 | B cams0-31 |
    B cams32-63), free = detection columns]; two detections (A,B) share each
    moving column via disjoint K-rows.
  - ACT subtracts 2^30 (exact power-of-two bias) -> Y = 64*k.
  - GPSIMD adds per-partition camera index j -> X = 64*k + j (exact fp32
    integers for any row-minimum; losers stay strictly larger).
  - DVE tensor_reduce(min, axis=X, apply_transpose=True) reduces each
    32-camera partition group into the free dim (32x32 stream transpose),
    all 128 lanes active.
  - Small TT-min combines the two 32-cam groups; int unpack j = X & 63,
    k = X >> 6; weights = valid/(1+sqrt(k/768)).
Host side only shards inputs, builds O(64) camera coefficient tables, and
un-permutes the outputs.
"""
import sys

sys.path.insert(0, "/opt/trn_rl_repo")

import numpy as np

N_TOTAL = 1 << 20
M_CAMS = 64
N_CORES = 8
N_C = N_TOTAL // N_CORES          # 131072 per core
HALF = N_C // 2                   # 65536 (A half / B half)
CH = 8192                         # moving columns per M-tile
NQ = HALF // CH                   # 8 M-tiles per core
REG = 2048                        # psum region columns
NREG = CH // REG                  # 4 regions per M-tile
S_FINE = 768.0
SS = 64.0 * S_FINE                # 49152
H_ROW = 2.0 ** 30
W_T = 0.3 / 3600.0
DC = 100.0                        # depth centering
TCEN = 1800.0                     # time centering
THRESH_K = 625.0 * S_FINE         # 480000

_CACHE = {}


def _build_module():
    import concourse.bacc as bacc
    import concourse.tile as tile
    from concourse import mybir

    f32 = mybir.dt.float32
    bf16 = mybir.dt.bfloat16
    i32 = mybir.dt.int32
    AF = mybir.ActivationFunctionType
    OP = mybir.AluOpType
    AX = mybir.AxisListType

    nc = bacc.Bacc("TRN2", target_bir_lowering=False)

    ddep = nc.dram_tensor("ddep", [N_C], f32, kind="ExternalInput")
    dtim = nc.dram_tensor("dtim", [N_C], f32, kind="ExternalInput")
    stat_in = nc.dram_tensor("stat", [40, 128], bf16, kind="ExternalInput")
    jb_in = nc.dram_tensor("jbias", [128, 1], f32, kind="ExternalInput")
    ones_in = nc.dram_tensor("ones4", [4, HALF], bf16, kind="ExternalInput")
    asn_out = nc.dram_tensor("asn", [64, 2048], i32, kind="ExternalOutput")
    wts_out = nc.dram_tensor("wts", [64, 2048], f32, kind="ExternalOutput")
    # bf16 moving-row scratch, columns = global detection index; 18 rows
    # (split-duplicated):
    #  0-5:  d'_0 d'_1 d'_2 d'_0 d'_1 d'_0   (coeff splits B0 B0 B0 B1 B1 B2)
    #  6-8:  d2_0 d2_1 d2_2                  (coeff SS exact)
    #  9-14: t_0 t_1 t_2 t_0 t_1 t_0         (coeff C0 C0 C0 C1 C1 C2)
    #  15-17: u_0 u_1 u_2                    (coeff SS exact)
    scratch = nc.dram_tensor("mscratch", [18, N_C], bf16)

    # scratch row lists per split index, offsets within a problem block
    DUP = {  # base -> {split -> [rows]}
        "d1": {0: [0, 3, 5], 1: [1, 4], 2: [2]},
        "d2": {0: [6], 1: [7], 2: [8]},
        "tau": {0: [9, 12, 14], 1: [10, 13], 2: [11]},
        "u": {0: [15], 1: [16], 2: [17]},
    }

    with tile.TileContext(nc) as tc:
        with (
            tc.tile_pool(name="const", bufs=1) as cpool,
            tc.tile_pool(name="prep", bufs=8) as prep,
            tc.tile_pool(name="load", bufs=2) as load,
            tc.tile_pool(name="mov", bufs=2) as mpool,
            tc.tile_pool(name="ps", bufs=2, space="PSUM") as ppool,
            tc.tile_pool(name="y", bufs=2) as ypool,
            tc.tile_pool(name="x", bufs=2) as xpool,
            tc.tile_pool(name="s1", bufs=1) as s1pool,
            tc.tile_pool(name="sh", bufs=1) as shpool,
            tc.tile_pool(name="post", bufs=1) as qpool,
        ):
            # ---- constants ----
            stat_t = cpool.tile([40, 128], bf16)
            nc.sync.dma_start(stat_t[:], stat_in[:])
            jb = cpool.tile([128, 1], f32)
            nc.sync.dma_start(jb[:], jb_in[:])
            bias_h = cpool.tile([128, 1], f32)
            nc.gpsimd.memset(bias_h[:], -(2.0 ** 30))
            bias_d = cpool.tile([128, 1], f32)
            nc.gpsimd.memset(bias_d[:], -DC)
            sw = float(np.float32(np.sqrt(W_T)))
            bias_t = cpool.tile([128, 1], f32)
            nc.gpsimd.memset(bias_t[:], -sw * TCEN)
            scale_t = cpool.tile([128, 1], f32)
            nc.gpsimd.memset(scale_t[:], sw)
            one_t = cpool.tile([128, 1], f32)
            nc.gpsimd.memset(one_t[:], 1.0)

            # ---- prep: f32 bases -> bf16 triple splits -> DRAM scratch ----
            dmaq = [nc.sync, nc.scalar]
            nd = [0]

            def wr(tile_, rows, hi):
                for r in rows:
                    eng = dmaq[nd[0] % 2]
                    nd[0] += 1
                    eng.dma_start(
                        scratch[r, hi * HALF:(hi + 1) * HALF].rearrange(
                            "(p f) -> p f", f=512),
                        tile_[:],
                    )

            for hi, h in enumerate(("A", "B")):
                off = 0 if h == "A" else HALF
                dload = load.tile([128, 512], f32, tag="dload")
                nc.sync.dma_start(
                    dload[:], ddep[off:off + HALF].rearrange("(p f) -> p f", f=512)
                )
                tload = load.tile([128, 512], f32, tag="tload")
                nc.scalar.dma_start(
                    tload[:], dtim[off:off + HALF].rearrange("(p f) -> p f", f=512)
                )
                bases = (
                    ("d1", dload, AF.Identity, one_t, bias_d),
                    ("d2", dload, AF.Square, one_t, bias_d),
                    ("tau", tload, AF.Identity, scale_t, bias_t),
                    ("u", tload, AF.Square, scale_t, bias_t),
                )
                for bn, srct, fn, sc, bias in bases:
                    x = prep.tile([128, 512], f32, tag="ppx")
                    nc.scalar.activation(x[:], srct[:], fn, bias=bias[:], scale=sc[:])
                    # triple split
                    s0 = prep.tile([128, 512], bf16, tag="pps0")
                    nc.vector.tensor_copy(s0[:], x[:])
                    r1 = prep.tile([128, 512], f32, tag="ppr1")
                    nc.gpsimd.tensor_tensor(out=r1[:], in0=x[:], in1=s0[:],
                                            op=OP.subtract)
                    s1t = prep.tile([128, 512], bf16, tag="pps1")
                    nc.vector.tensor_copy(s1t[:], r1[:])
                    r2 = prep.tile([128, 512], f32, tag="ppr2")
                    nc.vector.tensor_tensor(out=r2[:], in0=r1[:], in1=s1t[:],
                                            op=OP.subtract)
                    s2t = prep.tile([128, 512], bf16, tag="pps2")
                    nc.vector.tensor_copy(s2t[:], r2[:])
                    wr(s0, DUP[bn][0], hi)
                    wr(s1t, DUP[bn][1], hi)
                    wr(s2t, DUP[bn][2], hi)

            s1 = s1pool.tile([128, NQ * NREG * 64], f32)  # [128, 2048]

            # ---- main loop over M-tiles ----
            for q in range(NQ):
                m = mpool.tile([40, CH], bf16, tag="m")
                # rows 0-17: dets [16384q, +8192); rows 18-35: next 8192 dets
                nc.sync.dma_start(
                    m[0:18, :], scratch[:, 2 * q * CH:2 * q * CH + CH]
                )
                nc.sync.dma_start(
                    m[18:36, :], scratch[:, 2 * q * CH + CH:2 * (q + 1) * CH]
                )
                nc.scalar.dma_start(m[36:40, :], ones_in[:, q * CH:(q + 1) * CH])

                for r in range(NREG):
                    ps = ppool.tile([128, REG], f32, tag="ps")
                    for c in range(REG // 512):
                        col = r * REG + c * 512
                        nc.tensor.matmul(
                            ps[:, c * 512:(c + 1) * 512],
                            stat_t[:],
                            m[:, col:col + 512],
                            start=True,
                            stop=True,
                        )
                    y = ypool.tile([128, REG], f32, tag="y")
                    nc.scalar.activation(y[:], ps[:], AF.Identity, bias=bias_h[:])
                    x = xpool.tile([128, REG], f32, tag="x")
                    eng = nc.vector if (q % 2 == 0 and r == 0) else nc.gpsimd
                    eng.tensor_scalar(
                        out=x[:], in0=y[:], scalar1=jb[:], scalar2=None, op0=OP.add
                    )
                    nc.vector.tensor_reduce(
                        out=s1[:, q * 256 + r * 64:q * 256 + (r + 1) * 64],
                        in_=x[:].rearrange("p (b j) -> p b j", j=32),
                        op=OP.min,
                        axis=AX.X,
                        apply_transpose=True,
                    )

            # ---- stage 2 + post, incremental per 1024-col chunk ----
            for ci in range(2):
                cs = slice(ci * 1024, (ci + 1) * 1024)
                shg1 = shpool.tile([64, 1024], f32, tag="shg1")
                nc.sync.dma_start(shg1[0:32, :], s1[32:64, cs])
                nc.scalar.dma_start(shg1[32:64, :], s1[96:128, cs])
                shb0 = shpool.tile([64, 1024], f32, tag="shb0")
                nc.sync.dma_start(shb0[32:64, :], s1[64:96, cs])
                fa = shpool.tile([64, 1024], f32, tag="fa")
                nc.vector.tensor_tensor(
                    out=fa[0:32, :], in0=s1[0:32, cs], in1=shg1[0:32, :], op=OP.min
                )
                nc.vector.tensor_tensor(
                    out=fa[32:64, :], in0=shb0[32:64, :], in1=shg1[32:64, :],
                    op=OP.min,
                )
                ui = qpool.tile([64, 1024], i32, tag="ui")
                nc.vector.tensor_copy(ui[:], fa[:])
                ji = qpool.tile([64, 1024], i32, tag="ji")
                nc.vector.tensor_scalar(
                    out=ji[:], in0=ui[:], scalar1=63, scalar2=None, op0=OP.bitwise_and
                )
                nc.vector.tensor_scalar(
                    out=ui[:], in0=ui[:], scalar1=6, scalar2=None,
                    op0=OP.arith_shift_right,
                )
                kf = qpool.tile([64, 1024], f32, tag="kf")
                nc.vector.tensor_copy(kf[:], ui[:])
                jf = qpool.tile([64, 1024], f32, tag="jf")
                nc.gpsimd.tensor_copy(jf[:], ji[:])
                valid = qpool.tile([64, 1024], f32, tag="valid")
                nc.gpsimd.tensor_scalar(
                    out=valid[:], in0=kf[:], scalar1=THRESH_K, scalar2=None,
                    op0=OP.is_lt,
                )
                # assignments = (jf + 1) * valid - 1
                nc.gpsimd.tensor_scalar(
                    out=jf[:], in0=jf[:], scalar1=1.0, scalar2=None, op0=OP.add
                )
                nc.gpsimd.tensor_tensor(out=jf[:], in0=jf[:], in1=valid[:],
                                        op=OP.mult)
                nc.gpsimd.tensor_scalar(
                    out=jf[:], in0=jf[:], scalar1=1.0, scalar2=None, op0=OP.subtract
                )
                nc.gpsimd.tensor_copy(ji[:], jf[:])
                nc.sync.dma_start(asn_out[:, cs], ji[:])
                # weights = valid / (1 + sqrt(kf/768))
                nc.vector.tensor_scalar(
                    out=kf[:], in0=kf[:], scalar1=1.0 / S_FINE, scalar2=None,
                    op0=OP.mult,
                )
                sq = qpool.tile([64, 1024], f32, tag="sq")
                nc.scalar.activation(sq[:], kf[:], AF.Sqrt)
                nc.vector.tensor_scalar(
                    out=sq[:], in0=sq[:], scalar1=1.0, scalar2=None, op0=OP.add
                )
                nc.vector.reciprocal(sq[:], sq[:])
                nc.vector.tensor_tensor(out=sq[:], in0=sq[:], in1=valid[:],
                                        op=OP.mult)
                nc.scalar.dma_start(wts_out[:, cs], sq[:])

    nc.compile()
    return nc


def _host_consts(camera_depths, camera_times):
    import ml_dtypes
    bf = ml_dtypes.bfloat16

    def split3(x):
        x = np.asarray(x, np.float32)
        x0 = x.astype(bf).astype(np.float32)
        r1 = (x - x0).astype(np.float32)
        x1 = r1.astype(bf).astype(np.float32)
        r2 = (r1 - x1).astype(np.float32)
        x2 = r2.astype(bf).astype(np.float32)
        return x0, x1, x2

    cd = np.asarray(camera_depths, np.float64)
    ct = np.asarray(camera_times, np.float64)
    sw = float(np.float32(np.sqrt(W_T)))
    c1 = cd - DC
    t2c = sw * ct - sw * TCEN
    L = 0.5 * (1.0 - np.exp(-0.045 * cd))
    A = (SS * (c1 * c1 + t2c * t2c + L)).astype(np.float32)
    B = (SS * (-2.0 * c1)).astype(np.float32)
    C = (SS * (-2.0 * t2c)).astype(np.float32)
    Bs, Cs, As = split3(B), split3(C), split3(A)

    stat = np.zeros((40, 128), np.float32)
    jb = np.zeros((128, 1), np.float32)
    # per-problem data-row coeff layout (matches DUP in _build_module):
    # rows 0-5: B0 B0 B0 B1 B1 B2 ; 6-8: SS ; 9-14: C0 C0 C0 C1 C1 C2 ; 15-17: SS
    for g in range(4):
        cams = np.arange(32) + 32 * (g % 2)
        cols = slice(32 * g, 32 * g + 32)
        base = 0 if g < 2 else 18
        coefs = [Bs[0], Bs[0], Bs[0], Bs[1], Bs[1], Bs[2]]
        for ri, cf in enumerate(coefs):
            stat[base + ri, cols] = cf[cams]
        for ri in (6, 7, 8):
            stat[base + ri, cols] = SS
        coefs = [Cs[0], Cs[0], Cs[0], Cs[1], Cs[1], Cs[2]]
        for ri, cf in enumerate(coefs):
            stat[base + 9 + ri, cols] = cf[cams]
        for ri in (15, 16, 17):
            stat[base + ri, cols] = SS
        stat[36, cols] = As[0][cams]
        stat[37, cols] = As[1][cams]
        stat[38, cols] = As[2][cams]
        jb[cols, 0] = cams
    stat[39, :] = H_ROW
    # wait: rows 0-5 pattern above must pair with data splits d_0 d_1 d_2 d_0 d_1 d_0
    stat_b = stat.astype(bf)
    ones = np.ones((4, HALF), bf)
    return stat_b, jb, ones


def _det_perm():
    """device (p, m) -> core-local detection index, flattened [64*2048]."""
    p = np.arange(64)[:, None]
    m = np.arange(2048)[None, :]
    blk = p // 32
    i = p % 32
    q = m >> 8
    r = (m >> 6) & 3
    b = m & 63
    n = 2048 * r + 32 * b + i
    det = 2 * CH * q + CH * blk + n
    return det.ravel()


def kernel(detection_depths, camera_depths, detection_times, camera_times):
    from concourse.bass_utils import run_bass_kernel_spmd

    if "nc" not in _CACHE:
        _CACHE["nc"] = _build_module()
        _CACHE["perm"] = _det_perm()
    nc = _CACHE["nc"]
    perm = _CACHE["perm"]

    dd = np.ascontiguousarray(np.asarray(detection_depths, np.float32))
    dt = np.ascontiguousarray(np.asarray(detection_times, np.float32))
    stat, jb, ones = _host_consts(camera_depths, camera_times)

    in_maps = []
    for c in range(N_CORES):
        sl = slice(c * N_C, (c + 1) * N_C)
        in_maps.append({
            "ddep": dd[sl].copy(),
            "dtim": dt[sl].copy(),
            "stat": stat,
            "jbias": jb,
            "ones4": ones,
        })
    results = run_bass_kernel_spmd(nc, in_maps, list(range(N_CORES))).results

    assignments = np.empty(N_TOTAL, np.int32)
    weights = np.empty(N_TOTAL, np.float32)
    for c in range(N_CORES):
        base = c * N_C
        a_loc = np.empty(N_C, np.int32)
        w_loc = np.empty(N_C, np.float32)
        a_loc[perm] = results[c]["asn"].ravel()
        w_loc[perm] = results[c]["wts"].ravel()
        assignments[base:base + N_C] = a_loc
        weights[base:base + N_C] = w_loc
    return assignments, weights


# revision 19
# speedup vs baseline: 764.2454x; 1.0748x over previous
"""DepthWeightedAssignment Trainium2 kernel.

Per-row (detection) argmin over 64 cameras of
  cost[i,j] = (d_i-c_j)^2 + 0.5*(1-exp(-0.045 c_j)) + 0.3*(t_i-t_j)^2/3600
plus threshold/weight postprocessing, sharded over 8 NeuronCores (N axis).

Device algorithm (per core, N_c = 131072 detections):
  - PE matmul computes V = fl(49152*cost + 2^30) with the 2^30 row
    accumulated last, so V - 2^30 = 64*k exactly, k = round(768*cost).
    Layout: PSUM [128 partitions = (A cams0-31 | A cams32-63 | B cams0-31 |
    B cams32-63), free = detection columns]; two detections (A,B) share each
    moving column via disjoint K-rows.
  - ACT subtracts 2^30 (exact power-of-two bias) -> Y = 64*k.
  - GPSIMD adds per-partition camera index j -> X = 64*k + j (exact fp32
    integers for any row-minimum; losers stay strictly larger).
  - DVE tensor_reduce(min, axis=X, apply_transpose=True) reduces each
    32-camera partition group into the free dim (32x32 stream transpose),
    all 128 lanes active.
  - Small TT-min combines the two 32-cam groups; int unpack j = X & 63,
    k = X >> 6; weights = valid/(1+sqrt(k/768)).
Host side only shards inputs, builds O(64) camera coefficient tables, and
un-permutes the outputs.
"""
import sys

sys.path.insert(0, "/opt/trn_rl_repo")

import numpy as np

N_TOTAL = 1 << 20
M_CAMS = 64
N_CORES = 8
N_C = N_TOTAL // N_CORES          # 131072 per core
HALF = N_C // 2                   # 65536 (A half / B half)
CH = 8192                         # moving columns per M-tile
NQ = HALF // CH                   # 8 M-tiles per core
REG = 2048                        # psum region columns
NREG = CH // REG                  # 4 regions per M-tile
S_FINE = 768.0
SS = 64.0 * S_FINE                # 49152
H_ROW = 2.0 ** 30
W_T = 0.3 / 3600.0
DC = 100.0                        # depth centering
TCEN = 1800.0                     # time centering
THRESH_K = 625.0 * S_FINE         # 480000

_CACHE = {}


def _build_module():
    import concourse.bacc as bacc
    import concourse.tile as tile
    from concourse import mybir

    f32 = mybir.dt.float32
    bf16 = mybir.dt.bfloat16
    i32 = mybir.dt.int32
    AF = mybir.ActivationFunctionType
    OP = mybir.AluOpType
    AX = mybir.AxisListType

    nc = bacc.Bacc("TRN2", target_bir_lowering=False)

    ddep = nc.dram_tensor("ddep", [N_C], f32, kind="ExternalInput")
    dtim = nc.dram_tensor("dtim", [N_C], f32, kind="ExternalInput")
    stat_in = nc.dram_tensor("stat", [40, 128], bf16, kind="ExternalInput")
    jb_in = nc.dram_tensor("jbias", [128, 1], f32, kind="ExternalInput")
    ones_in = nc.dram_tensor("ones4", [4, HALF], bf16, kind="ExternalInput")
    asn_out = nc.dram_tensor("asn", [64, 2048], i32, kind="ExternalOutput")
    wts_out = nc.dram_tensor("wts", [64, 2048], f32, kind="ExternalOutput")
    # bf16 moving-row scratch, columns = global detection index; 18 rows
    # (split-duplicated):
    #  0-5:  d'_0 d'_1 d'_2 d'_0 d'_1 d'_0   (coeff splits B0 B0 B0 B1 B1 B2)
    #  6-8:  d2_0 d2_1 d2_2                  (coeff SS exact)
    #  9-14: t_0 t_1 t_2 t_0 t_1 t_0         (coeff C0 C0 C0 C1 C1 C2)
    #  15-17: u_0 u_1 u_2                    (coeff SS exact)
    scratch = nc.dram_tensor("mscratch", [18, N_C], bf16)

    # scratch row lists per split index, offsets within a problem block
    DUP = {  # base -> {split -> [rows]}
        "d1": {0: [0, 3, 5], 1: [1, 4], 2: [2]},
        "d2": {0: [6], 1: [7], 2: [8]},
        "tau": {0: [9, 12, 14], 1: [10, 13], 2: [11]},
        "u": {0: [15], 1: [16], 2: [17]},
    }

    with tile.TileContext(nc) as tc:
        with (
            tc.tile_pool(name="const", bufs=1) as cpool,
            tc.tile_pool(name="prep", bufs=4) as prep,
            tc.tile_pool(name="load", bufs=2) as load,
            tc.tile_pool(name="mov", bufs=3) as mpool,
            tc.tile_pool(name="ps", bufs=2, space="PSUM") as ppool,
            tc.tile_pool(name="y", bufs=4) as ypool,
            tc.tile_pool(name="x", bufs=4) as xpool,
            tc.tile_pool(name="s1", bufs=1) as s1pool,
            tc.tile_pool(name="sh", bufs=1) as shpool,
            tc.tile_pool(name="post", bufs=1) as qpool,
        ):
            # ---- constants ----
            stat_t = cpool.tile([40, 128], bf16)
            nc.sync.dma_start(stat_t[:], stat_in[:])
            jb = cpool.tile([128, 1], f32)
            nc.sync.dma_start(jb[:], jb_in[:])
            bias_h = cpool.tile([128, 1], f32)
            nc.gpsimd.memset(bias_h[:], -(2.0 ** 30))
            bias_d = cpool.tile([128, 1], f32)
            nc.gpsimd.memset(bias_d[:], -DC)
            sw = float(np.float32(np.sqrt(W_T)))
            bias_t = cpool.tile([128, 1], f32)
            nc.gpsimd.memset(bias_t[:], -sw * TCEN)
            scale_t = cpool.tile([128, 1], f32)
            nc.gpsimd.memset(scale_t[:], sw)
            one_t = cpool.tile([128, 1], f32)
            nc.gpsimd.memset(one_t[:], 1.0)

            # ---- prep: f32 bases -> bf16 triple splits -> DRAM scratch ----
            dmaq = [nc.sync, nc.scalar]
            nd = [0]

            def wr(tile_, rows, hi):
                for r in rows:
                    eng = dmaq[nd[0] % 2]
                    nd[0] += 1
                    eng.dma_start(
                        scratch[r, hi * HALF:(hi + 1) * HALF].rearrange(
                            "(p f) -> p f", f=512),
                        tile_[:],
                    )

            for hi, h in enumerate(("A", "B")):
                off = 0 if h == "A" else HALF
                dload = load.tile([128, 512], f32, tag="dload")
                nc.sync.dma_start(
                    dload[:], ddep[off:off + HALF].rearrange("(p f) -> p f", f=512)
                )
                tload = load.tile([128, 512], f32, tag="tload")
                nc.scalar.dma_start(
                    tload[:], dtim[off:off + HALF].rearrange("(p f) -> p f", f=512)
                )
                bases = (
                    ("d1", dload, AF.Identity, one_t, bias_d),
                    ("d2", dload, AF.Square, one_t, bias_d),
                    ("tau", tload, AF.Identity, scale_t, bias_t),
                    ("u", tload, AF.Square, scale_t, bias_t),
                )
                for bn, srct, fn, sc, bias in bases:
                    x = prep.tile([128, 512], f32, tag="ppx")
                    nc.scalar.activation(x[:], srct[:], fn, bias=bias[:], scale=sc[:])
                    # triple split
                    s0 = prep.tile([128, 512], bf16, tag="pps0")
                    nc.vector.tensor_copy(s0[:], x[:])
                    r1 = prep.tile([128, 512], f32, tag="ppr1")
                    nc.gpsimd.tensor_tensor(out=r1[:], in0=x[:], in1=s0[:],
                                            op=OP.subtract)
                    s1t = prep.tile([128, 512], bf16, tag="pps1")
                    nc.vector.tensor_copy(s1t[:], r1[:])
                    r2 = prep.tile([128, 512], f32, tag="ppr2")
                    nc.vector.tensor_tensor(out=r2[:], in0=r1[:], in1=s1t[:],
                                            op=OP.subtract)
                    s2t = prep.tile([128, 512], bf16, tag="pps2")
                    nc.vector.tensor_copy(s2t[:], r2[:])
                    wr(s0, DUP[bn][0], hi)
                    wr(s1t, DUP[bn][1], hi)
                    wr(s2t, DUP[bn][2], hi)

            s1 = s1pool.tile([128, NQ * NREG * 64], f32)  # [128, 2048]

            # ---- main loop over M-tiles ----
            for q in range(NQ):
                m = mpool.tile([40, CH], bf16, tag="m")
                # rows 0-17: dets [16384q, +8192); rows 18-35: next 8192 dets
                nc.sync.dma_start(
                    m[0:18, :], scratch[:, 2 * q * CH:2 * q * CH + CH]
                )
                nc.sync.dma_start(
                    m[18:36, :], scratch[:, 2 * q * CH + CH:2 * (q + 1) * CH]
                )
                nc.scalar.dma_start(m[36:40, :], ones_in[:, q * CH:(q + 1) * CH])

                for r in range(NREG):
                    ps = ppool.tile([128, REG], f32, tag="ps")
                    for c in range(REG // 512):
                        col = r * REG + c * 512
                        nc.tensor.matmul(
                            ps[:, c * 512:(c + 1) * 512],
                            stat_t[:],
                            m[:, col:col + 512],
                            start=True,
                            stop=True,
                        )
                    y = ypool.tile([128, REG], f32, tag="y")
                    nc.scalar.activation(y[:], ps[:], AF.Identity, bias=bias_h[:])
                    x = xpool.tile([128, REG], f32, tag="x")
                    eng = nc.vector if (q % 2 == 0 and r == 0) else nc.gpsimd
                    eng.tensor_scalar(
                        out=x[:], in0=y[:], scalar1=jb[:], scalar2=None, op0=OP.add
                    )
                    nc.vector.tensor_reduce(
                        out=s1[:, q * 256 + r * 64:q * 256 + (r + 1) * 64],
                        in_=x[:].rearrange("p (b j) -> p b j", j=32),
                        op=OP.min,
                        axis=AX.X,
                        apply_transpose=True,
                    )

            # ---- stage 2 + post, incremental per 1024-col chunk ----
            for ci in range(2):
                cs = slice(ci * 1024, (ci + 1) * 1024)
                shg1 = shpool.tile([64, 1024], f32, tag="shg1")
                nc.sync.dma_start(shg1[0:32, :], s1[32:64, cs])
                nc.scalar.dma_start(shg1[32:64, :], s1[96:128, cs])
                shb0 = shpool.tile([64, 1024], f32, tag="shb0")
                nc.sync.dma_start(shb0[32:64, :], s1[64:96, cs])
                fa = shpool.tile([64, 1024], f32, tag="fa")
                nc.vector.tensor_tensor(
                    out=fa[0:32, :], in0=s1[0:32, cs], in1=shg1[0:32, :], op=OP.min
                )
                nc.vector.tensor_tensor(
                    out=fa[32:64, :], in0=shb0[32:64, :], in1=shg1[32:64, :],
                    op=OP.min,
                )
                ui = qpool.tile([64, 1024], i32, tag="ui")
                nc.vector.tensor_copy(ui[:], fa[:])
                ji = qpool.tile([64, 1024], i32, tag="ji")
                nc.vector.tensor_scalar(
                    out=ji[:], in0=ui[:], scalar1=63, scalar2=None, op0=OP.bitwise_and
                )
                nc.vector.tensor_scalar(
                    out=ui[:], in0=ui[:], scalar1=6, scalar2=None,
                    op0=OP.arith_shift_right,
                )
                kf = qpool.tile([64, 1024], f32, tag="kf")
                nc.vector.tensor_copy(kf[:], ui[:])
                jf = qpool.tile([64, 1024], f32, tag="jf")
                nc.gpsimd.tensor_copy(jf[:], ji[:])
                valid = qpool.tile([64, 1024], f32, tag="valid")
                nc.gpsimd.tensor_scalar(
                    out=valid[:], in0=kf[:], scalar1=THRESH_K, scalar2=None,
                    op0=OP.is_lt,
                )
                # assignments = (jf + 1) * valid - 1
                nc.gpsimd.tensor_scalar(
                    out=jf[:], in0=jf[:], scalar1=1.0, scalar2=None, op0=OP.add
                )
                nc.gpsimd.tensor_tensor(out=jf[:], in0=jf[:], in1=valid[:],
                                        op=OP.mult)
                nc.gpsimd.tensor_scalar(
                    out=jf[:], in0=jf[:], scalar1=1.0, scalar2=None, op0=OP.subtract
                )
                nc.gpsimd.tensor_copy(ji[:], jf[:])
                nc.sync.dma_start(asn_out[:, cs], ji[:])
                # weights = valid / (1 + sqrt(kf/768))
                nc.vector.tensor_scalar(
                    out=kf[:], in0=kf[:], scalar1=1.0 / S_FINE, scalar2=None,
                    op0=OP.mult,
                )
                sq = qpool.tile([64, 1024], f32, tag="sq")
                nc.scalar.activation(sq[:], kf[:], AF.Sqrt)
                nc.vector.tensor_scalar(
                    out=sq[:], in0=sq[:], scalar1=1.0, scalar2=None, op0=OP.add
                )
                nc.vector.reciprocal(sq[:], sq[:])
                nc.vector.tensor_tensor(out=sq[:], in0=sq[:], in1=valid[:],
                                        op=OP.mult)
                nc.scalar.dma_start(wts_out[:, cs], sq[:])

    nc.compile()
    return nc


def _host_consts(camera_depths, camera_times):
    import ml_dtypes
    bf = ml_dtypes.bfloat16

    def split3(x):
        x = np.asarray(x, np.float32)
        x0 = x.astype(bf).astype(np.float32)
        r1 = (x - x0).astype(np.float32)
        x1 = r1.astype(bf).astype(np.float32)
        r2 = (r1 - x1).astype(np.float32)
        x2 = r2.astype(bf).astype(np.float32)
        return x0, x1, x2

    cd = np.asarray(camera_depths, np.float64)
    ct = np.asarray(camera_times, np.float64)
    sw = float(np.float32(np.sqrt(W_T)))
    c1 = cd - DC
    t2c = sw * ct - sw * TCEN
    L = 0.5 * (1.0 - np.exp(-0.045 * cd))
    A = (SS * (c1 * c1 + t2c * t2c + L)).astype(np.float32)
    B = (SS * (-2.0 * c1)).astype(np.float32)
    C = (SS * (-2.0 * t2c)).astype(np.float32)
    Bs, Cs, As = split3(B), split3(C), split3(A)

    stat = np.zeros((40, 128), np.float32)
    jb = np.zeros((128, 1), np.float32)
    # per-problem data-row coeff layout (matches DUP in _build_module):
    # rows 0-5: B0 B0 B0 B1 B1 B2 ; 6-8: SS ; 9-14: C0 C0 C0 C1 C1 C2 ; 15-17: SS
    for g in range(4):
        cams = np.arange(32) + 32 * (g % 2)
        cols = slice(32 * g, 32 * g + 32)
        base = 0 if g < 2 else 18
        coefs = [Bs[0], Bs[0], Bs[0], Bs[1], Bs[1], Bs[2]]
        for ri, cf in enumerate(coefs):
            stat[base + ri, cols] = cf[cams]
        for ri in (6, 7, 8):
            stat[base + ri, cols] = SS
        coefs = [Cs[0], Cs[0], Cs[0], Cs[1], Cs[1], Cs[2]]
        for ri, cf in enumerate(coefs):
            stat[base + 9 + ri, cols] = cf[cams]
        for ri in (15, 16, 17):
            stat[base + ri, cols] = SS
        stat[36, cols] = As[0][cams]
        stat[37, cols] = As[1][cams]
        stat[38, cols] = As[2][cams]
        jb[cols, 0] = cams
    stat[39, :] = H_ROW
    # wait: rows 0-5 pattern above must pair with data splits d_0 d_1 d_2 d_0 d_1 d_0
    stat_b = stat.astype(bf)
    ones = np.ones((4, HALF), bf)
    return stat_b, jb, ones


def _det_perm():
    """device (p, m) -> core-local detection index, flattened [64*2048]."""
    p = np.arange(64)[:, None]
    m = np.arange(2048)[None, :]
    blk = p // 32
    i = p % 32
    q = m >> 8
    r = (m >> 6) & 3
    b = m & 63
    n = 2048 * r + 32 * b + i
    det = 2 * CH * q + CH * blk + n
    return det.ravel()


def kernel(detection_depths, camera_depths, detection_times, camera_times):
    from concourse.bass_utils import run_bass_kernel_spmd

    if "nc" not in _CACHE:
        _CACHE["nc"] = _build_module()
        _CACHE["perm"] = _det_perm()
    nc = _CACHE["nc"]
    perm = _CACHE["perm"]

    dd = np.ascontiguousarray(np.asarray(detection_depths, np.float32))
    dt = np.ascontiguousarray(np.asarray(detection_times, np.float32))
    stat, jb, ones = _host_consts(camera_depths, camera_times)

    in_maps = []
    for c in range(N_CORES):
        sl = slice(c * N_C, (c + 1) * N_C)
        in_maps.append({
            "ddep": dd[sl].copy(),
            "dtim": dt[sl].copy(),
            "stat": stat,
            "jbias": jb,
            "ones4": ones,
        })
    results = run_bass_kernel_spmd(nc, in_maps, list(range(N_CORES))).results

    assignments = np.empty(N_TOTAL, np.int32)
    weights = np.empty(N_TOTAL, np.float32)
    for c in range(N_CORES):
        base = c * N_C
        a_loc = np.empty(N_C, np.int32)
        w_loc = np.empty(N_C, np.float32)
        a_loc[perm] = results[c]["asn"].ravel()
        w_loc[perm] = results[c]["wts"].ravel()
        assignments[base:base + N_C] = a_loc
        weights[base:base + N_C] = w_loc
    return assignments, weights


# revision 21
# speedup vs baseline: 779.8849x; 1.0205x over previous
"""DepthWeightedAssignment Trainium2 kernel.

Per-row (detection) argmin over 64 cameras of
  cost[i,j] = (d_i-c_j)^2 + 0.5*(1-exp(-0.045 c_j)) + 0.3*(t_i-t_j)^2/3600
plus threshold/weight postprocessing, sharded over 8 NeuronCores (N axis).

Device algorithm (per core, N_c = 131072 detections):
  - PE matmul computes V = fl(49152*cost + 2^30) with the 2^30 row
    accumulated last, so V - 2^30 = 64*k exactly, k = round(768*cost).
    Layout: PSUM [128 partitions = (A cams0-31 | A cams32-63 | B cams0-31 |
    B cams32-63), free = detection columns]; two detections (A,B) share each
    moving column via disjoint K-rows.
  - ACT subtracts 2^30 (exact power-of-two bias) -> Y = 64*k.
  - GPSIMD adds per-partition camera index j -> X = 64*k + j (exact fp32
    integers for any row-minimum; losers stay strictly larger).
  - DVE tensor_reduce(min, axis=X, apply_transpose=True) reduces each
    32-camera partition group into the free dim (32x32 stream transpose),
    all 128 lanes active.
  - Small TT-min combines the two 32-cam groups; int unpack j = X & 63,
    k = X >> 6; weights = valid/(1+sqrt(k/768)).
Host side only shards inputs, builds O(64) camera coefficient tables, and
un-permutes the outputs.
"""
import sys

sys.path.insert(0, "/opt/trn_rl_repo")

import numpy as np

N_TOTAL = 1 << 20
M_CAMS = 64
N_CORES = 8
N_C = N_TOTAL // N_CORES          # 131072 per core
HALF = N_C // 2                   # 65536 (A half / B half)
CH = 8192                         # moving columns per M-tile
NQ = HALF // CH                   # 8 M-tiles per core
REG = 2048                        # psum region columns
NREG = CH // REG                  # 4 regions per M-tile
S_FINE = 768.0
SS = 64.0 * S_FINE                # 49152
H_ROW = 2.0 ** 30
W_T = 0.3 / 3600.0
DC = 100.0                        # depth centering
TCEN = 1800.0                     # time centering
THRESH_K = 625.0 * S_FINE         # 480000

_CACHE = {}


def _build_module():
    import concourse.bacc as bacc
    import concourse.tile as tile
    from concourse import mybir

    f32 = mybir.dt.float32
    bf16 = mybir.dt.bfloat16
    i32 = mybir.dt.int32
    AF = mybir.ActivationFunctionType
    OP = mybir.AluOpType
    AX = mybir.AxisListType

    nc = bacc.Bacc("TRN2", target_bir_lowering=False)

    ddep = nc.dram_tensor("ddep", [N_C], f32, kind="ExternalInput")
    dtim = nc.dram_tensor("dtim", [N_C], f32, kind="ExternalInput")
    stat_in = nc.dram_tensor("stat", [40, 128], bf16, kind="ExternalInput")
    jb_in = nc.dram_tensor("jbias", [128, 1], f32, kind="ExternalInput")
    ones_in = nc.dram_tensor("ones4", [4, HALF], bf16, kind="ExternalInput")
    asn_out = nc.dram_tensor("asn", [64, 2048], i32, kind="ExternalOutput")
    wts_out = nc.dram_tensor("wts", [64, 2048], f32, kind="ExternalOutput")
    # bf16 moving-row scratch, columns = global detection index; 18 rows
    # (split-duplicated):
    #  0-5:  d'_0 d'_1 d'_2 d'_0 d'_1 d'_0   (coeff splits B0 B0 B0 B1 B1 B2)
    #  6-8:  d2_0 d2_1 d2_2                  (coeff SS exact)
    #  9-14: t_0 t_1 t_2 t_0 t_1 t_0         (coeff C0 C0 C0 C1 C1 C2)
    #  15-17: u_0 u_1 u_2                    (coeff SS exact)
    scratch = nc.dram_tensor("mscratch", [18, N_C], bf16)

    # scratch row lists per split index, offsets within a problem block
    DUP = {  # base -> {split -> [rows]}
        "d1": {0: [0, 3, 5], 1: [1, 4], 2: [2]},
        "d2": {0: [6], 1: [7], 2: [8]},
        "tau": {0: [9, 12, 14], 1: [10, 13], 2: [11]},
        "u": {0: [15], 1: [16], 2: [17]},
    }

    with tile.TileContext(nc) as tc:
        with (
            tc.tile_pool(name="const", bufs=1) as cpool,
            tc.tile_pool(name="prep", bufs=4) as prep,
            tc.tile_pool(name="load", bufs=2) as load,
            tc.tile_pool(name="mov", bufs=3) as mpool,
            tc.tile_pool(name="ps", bufs=2, space="PSUM") as ppool,
            tc.tile_pool(name="y", bufs=4) as ypool,
            tc.tile_pool(name="x", bufs=4) as xpool,
            tc.tile_pool(name="s1", bufs=1) as s1pool,
            tc.tile_pool(name="sh", bufs=2) as shpool,
            tc.tile_pool(name="post", bufs=2) as qpool,
        ):
            # ---- constants ----
            stat_t = cpool.tile([40, 128], bf16)
            nc.sync.dma_start(stat_t[:], stat_in[:])
            jb = cpool.tile([128, 1], f32)
            nc.sync.dma_start(jb[:], jb_in[:])
            bias_h = cpool.tile([128, 1], f32)
            nc.gpsimd.memset(bias_h[:], -(2.0 ** 30))
            bias_d = cpool.tile([128, 1], f32)
            nc.gpsimd.memset(bias_d[:], -DC)
            sw = float(np.float32(np.sqrt(W_T)))
            bias_t = cpool.tile([128, 1], f32)
            nc.gpsimd.memset(bias_t[:], -sw * TCEN)
            scale_t = cpool.tile([128, 1], f32)
            nc.gpsimd.memset(scale_t[:], sw)
            one_t = cpool.tile([128, 1], f32)
            nc.gpsimd.memset(one_t[:], 1.0)

            # ---- prep: f32 bases -> bf16 triple splits -> DRAM scratch ----
            dmaq = [nc.sync, nc.scalar]
            nd = [0]

            def wr(tile_, rows, hi):
                for r in rows:
                    eng = dmaq[nd[0] % 2]
                    nd[0] += 1
                    eng.dma_start(
                        scratch[r, hi * HALF:(hi + 1) * HALF].rearrange(
                            "(p f) -> p f", f=512),
                        tile_[:],
                    )

            for hi, h in enumerate(("A", "B")):
                off = 0 if h == "A" else HALF
                dload = load.tile([128, 512], f32, tag="dload")
                nc.sync.dma_start(
                    dload[:], ddep[off:off + HALF].rearrange("(p f) -> p f", f=512)
                )
                tload = load.tile([128, 512], f32, tag="tload")
                nc.scalar.dma_start(
                    tload[:], dtim[off:off + HALF].rearrange("(p f) -> p f", f=512)
                )
                bases = (
                    ("d1", dload, AF.Identity, one_t, bias_d),
                    ("d2", dload, AF.Square, one_t, bias_d),
                    ("tau", tload, AF.Identity, scale_t, bias_t),
                    ("u", tload, AF.Square, scale_t, bias_t),
                )
                for bn, srct, fn, sc, bias in bases:
                    x = prep.tile([128, 512], f32, tag="ppx")
                    nc.scalar.activation(x[:], srct[:], fn, bias=bias[:], scale=sc[:])
                    # triple split
                    s0 = prep.tile([128, 512], bf16, tag="pps0")
                    nc.vector.tensor_copy(s0[:], x[:])
                    r1 = prep.tile([128, 512], f32, tag="ppr1")
                    nc.gpsimd.tensor_tensor(out=r1[:], in0=x[:], in1=s0[:],
                                            op=OP.subtract)
                    s1t = prep.tile([128, 512], bf16, tag="pps1")
                    nc.vector.tensor_copy(s1t[:], r1[:])
                    r2 = prep.tile([128, 512], f32, tag="ppr2")
                    nc.vector.tensor_tensor(out=r2[:], in0=r1[:], in1=s1t[:],
                                            op=OP.subtract)
                    s2t = prep.tile([128, 512], bf16, tag="pps2")
                    nc.vector.tensor_copy(s2t[:], r2[:])
                    wr(s0, DUP[bn][0], hi)
                    wr(s1t, DUP[bn][1], hi)
                    wr(s2t, DUP[bn][2], hi)

            s1 = s1pool.tile([128, NQ * NREG * 64], f32)  # [128, 2048]

            # ---- main loop over M-tiles ----
            for q in range(NQ):
                m = mpool.tile([40, CH], bf16, tag="m")
                # rows 0-17: dets [16384q, +8192); rows 18-35: next 8192 dets
                nc.sync.dma_start(
                    m[0:18, :], scratch[:, 2 * q * CH:2 * q * CH + CH]
                )
                nc.sync.dma_start(
                    m[18:36, :], scratch[:, 2 * q * CH + CH:2 * (q + 1) * CH]
                )
                nc.scalar.dma_start(m[36:40, :], ones_in[:, q * CH:(q + 1) * CH])

                for r in range(NREG):
                    ps = ppool.tile([128, REG], f32, tag="ps")
                    for c in range(REG // 512):
                        col = r * REG + c * 512
                        nc.tensor.matmul(
                            ps[:, c * 512:(c + 1) * 512],
                            stat_t[:],
                            m[:, col:col + 512],
                            start=True,
                            stop=True,
                        )
                    y = ypool.tile([128, REG], f32, tag="y")
                    nc.scalar.activation(y[:], ps[:], AF.Identity, bias=bias_h[:])
                    x = xpool.tile([128, REG], f32, tag="x")
                    eng = nc.vector if (q % 2 == 0 and r == 0) else nc.gpsimd
                    eng.tensor_scalar(
                        out=x[:], in0=y[:], scalar1=jb[:], scalar2=None, op0=OP.add
                    )
                    nc.vector.tensor_reduce(
                        out=s1[:, q * 256 + r * 64:q * 256 + (r + 1) * 64],
                        in_=x[:].rearrange("p (b j) -> p b j", j=32),
                        op=OP.min,
                        axis=AX.X,
                        apply_transpose=True,
                    )

            # ---- stage 2 + post, incremental per 512-col chunk ----
            for ci in range(4):
                cs = slice(ci * 512, (ci + 1) * 512)
                shg1 = shpool.tile([64, 512], f32, tag="shg1")
                nc.sync.dma_start(shg1[0:32, :], s1[32:64, cs])
                nc.scalar.dma_start(shg1[32:64, :], s1[96:128, cs])
                shb0 = shpool.tile([64, 512], f32, tag="shb0")
                nc.sync.dma_start(shb0[32:64, :], s1[64:96, cs])
                fa = shpool.tile([64, 512], f32, tag="fa")
                nc.vector.tensor_tensor(
                    out=fa[0:32, :], in0=s1[0:32, cs], in1=shg1[0:32, :], op=OP.min
                )
                nc.vector.tensor_tensor(
                    out=fa[32:64, :], in0=shb0[32:64, :], in1=shg1[32:64, :],
                    op=OP.min,
                )
                ui = qpool.tile([64, 512], i32, tag="ui")
                nc.vector.tensor_copy(ui[:], fa[:])
                ji = qpool.tile([64, 512], i32, tag="ji")
                nc.vector.tensor_scalar(
                    out=ji[:], in0=ui[:], scalar1=63, scalar2=None, op0=OP.bitwise_and
                )
                nc.vector.tensor_scalar(
                    out=ui[:], in0=ui[:], scalar1=6, scalar2=None,
                    op0=OP.arith_shift_right,
                )
                kf = qpool.tile([64, 512], f32, tag="kf")
                nc.vector.tensor_copy(kf[:], ui[:])
                jf = qpool.tile([64, 512], f32, tag="jf")
                nc.gpsimd.tensor_copy(jf[:], ji[:])
                valid = qpool.tile([64, 512], f32, tag="valid")
                nc.gpsimd.tensor_scalar(
                    out=valid[:], in0=kf[:], scalar1=THRESH_K, scalar2=None,
                    op0=OP.is_lt,
                )
                # assignments = (jf + 1) * valid - 1
                nc.gpsimd.tensor_scalar(
                    out=jf[:], in0=jf[:], scalar1=1.0, scalar2=None, op0=OP.add
                )
                nc.gpsimd.tensor_tensor(out=jf[:], in0=jf[:], in1=valid[:],
                                        op=OP.mult)
                nc.gpsimd.tensor_scalar(
                    out=jf[:], in0=jf[:], scalar1=1.0, scalar2=None, op0=OP.subtract
                )
                nc.gpsimd.tensor_copy(ji[:], jf[:])
                nc.sync.dma_start(asn_out[:, cs], ji[:])
                # weights = valid / (1 + sqrt(kf/768))
                nc.vector.tensor_scalar(
                    out=kf[:], in0=kf[:], scalar1=1.0 / S_FINE, scalar2=None,
                    op0=OP.mult,
                )
                sq = qpool.tile([64, 512], f32, tag="sq")
                nc.scalar.activation(sq[:], kf[:], AF.Sqrt)
                nc.vector.tensor_scalar(
                    out=sq[:], in0=sq[:], scalar1=1.0, scalar2=None, op0=OP.add
                )
                nc.vector.reciprocal(sq[:], sq[:])
                nc.vector.tensor_tensor(out=sq[:], in0=sq[:], in1=valid[:],
                                        op=OP.mult)
                nc.scalar.dma_start(wts_out[:, cs], sq[:])

    nc.compile()
    return nc


def _host_consts(camera_depths, camera_times):
    import ml_dtypes
    bf = ml_dtypes.bfloat16

    def split3(x):
        x = np.asarray(x, np.float32)
        x0 = x.astype(bf).astype(np.float32)
        r1 = (x - x0).astype(np.float32)
        x1 = r1.astype(bf).astype(np.float32)
        r2 = (r1 - x1).astype(np.float32)
        x2 = r2.astype(bf).astype(np.float32)
        return x0, x1, x2

    cd = np.asarray(camera_depths, np.float64)
    ct = np.asarray(camera_times, np.float64)
    sw = float(np.float32(np.sqrt(W_T)))
    c1 = cd - DC
    t2c = sw * ct - sw * TCEN
    L = 0.5 * (1.0 - np.exp(-0.045 * cd))
    A = (SS * (c1 * c1 + t2c * t2c + L)).astype(np.float32)
    B = (SS * (-2.0 * c1)).astype(np.float32)
    C = (SS * (-2.0 * t2c)).astype(np.float32)
    Bs, Cs, As = split3(B), split3(C), split3(A)

    stat = np.zeros((40, 128), np.float32)
    jb = np.zeros((128, 1), np.float32)
    # per-problem data-row coeff layout (matches DUP in _build_module):
    # rows 0-5: B0 B0 B0 B1 B1 B2 ; 6-8: SS ; 9-14: C0 C0 C0 C1 C1 C2 ; 15-17: SS
    for g in range(4):
        cams = np.arange(32) + 32 * (g % 2)
        cols = slice(32 * g, 32 * g + 32)
        base = 0 if g < 2 else 18
        coefs = [Bs[0], Bs[0], Bs[0], Bs[1], Bs[1], Bs[2]]
        for ri, cf in enumerate(coefs):
            stat[base + ri, cols] = cf[cams]
        for ri in (6, 7, 8):
            stat[base + ri, cols] = SS
        coefs = [Cs[0], Cs[0], Cs[0], Cs[1], Cs[1], Cs[2]]
        for ri, cf in enumerate(coefs):
            stat[base + 9 + ri, cols] = cf[cams]
        for ri in (15, 16, 17):
            stat[base + ri, cols] = SS
        stat[36, cols] = As[0][cams]
        stat[37, cols] = As[1][cams]
        stat[38, cols] = As[2][cams]
        jb[cols, 0] = cams
    stat[39, :] = H_ROW
    # wait: rows 0-5 pattern above must pair with data splits d_0 d_1 d_2 d_0 d_1 d_0
    stat_b = stat.astype(bf)
    ones = np.ones((4, HALF), bf)
    return stat_b, jb, ones


def _det_perm():
    """device (p, m) -> core-local detection index, flattened [64*2048]."""
    p = np.arange(64)[:, None]
    m = np.arange(2048)[None, :]
    blk = p // 32
    i = p % 32
    q = m >> 8
    r = (m >> 6) & 3
    b = m & 63
    n = 2048 * r + 32 * b + i
    det = 2 * CH * q + CH * blk + n
    return det.ravel()


def kernel(detection_depths, camera_depths, detection_times, camera_times):
    from concourse.bass_utils import run_bass_kernel_spmd

    if "nc" not in _CACHE:
        _CACHE["nc"] = _build_module()
        _CACHE["perm"] = _det_perm()
    nc = _CACHE["nc"]
    perm = _CACHE["perm"]

    dd = np.ascontiguousarray(np.asarray(detection_depths, np.float32))
    dt = np.ascontiguousarray(np.asarray(detection_times, np.float32))
    stat, jb, ones = _host_consts(camera_depths, camera_times)

    in_maps = []
    for c in range(N_CORES):
        sl = slice(c * N_C, (c + 1) * N_C)
        in_maps.append({
            "ddep": dd[sl].copy(),
            "dtim": dt[sl].copy(),
            "stat": stat,
            "jbias": jb,
            "ones4": ones,
        })
    results = run_bass_kernel_spmd(nc, in_maps, list(range(N_CORES))).results

    assignments = np.empty(N_TOTAL, np.int32)
    weights = np.empty(N_TOTAL, np.float32)
    for c in range(N_CORES):
        base = c * N_C
        a_loc = np.empty(N_C, np.int32)
        w_loc = np.empty(N_C, np.float32)
        a_loc[perm] = results[c]["asn"].ravel()
        w_loc[perm] = results[c]["wts"].ravel()
        assignments[base:base + N_C] = a_loc
        weights[base:base + N_C] = w_loc
    return assignments, weights


# revision 22
# speedup vs baseline: 800.8557x; 1.0269x over previous
"""DepthWeightedAssignment Trainium2 kernel.

Per-row (detection) argmin over 64 cameras of
  cost[i,j] = (d_i-c_j)^2 + 0.5*(1-exp(-0.045 c_j)) + 0.3*(t_i-t_j)^2/3600
plus threshold/weight postprocessing, sharded over 8 NeuronCores (N axis).

Device algorithm (per core, N_c = 131072 detections):
  - PE matmul computes V = fl(49152*cost + 2^30) with the 2^30 row
    accumulated last, so V - 2^30 = 64*k exactly, k = round(768*cost).
    Layout: PSUM [128 partitions = (A cams0-31 | A cams32-63 | B cams0-31 |
    B cams32-63), free = detection columns]; two detections (A,B) share each
    moving column via disjoint K-rows.
  - ACT subtracts 2^30 (exact power-of-two bias) -> Y = 64*k.
  - GPSIMD adds per-partition camera index j -> X = 64*k + j (exact fp32
    integers for any row-minimum; losers stay strictly larger).
  - DVE tensor_reduce(min, axis=X, apply_transpose=True) reduces each
    32-camera partition group into the free dim (32x32 stream transpose),
    all 128 lanes active.
  - Small TT-min combines the two 32-cam groups; int unpack j = X & 63,
    k = X >> 6; weights = valid/(1+sqrt(k/768)).
Host side only shards inputs, builds O(64) camera coefficient tables, and
un-permutes the outputs.
"""
import sys

sys.path.insert(0, "/opt/trn_rl_repo")

import numpy as np

N_TOTAL = 1 << 20
M_CAMS = 64
N_CORES = 8
N_C = N_TOTAL // N_CORES          # 131072 per core
HALF = N_C // 2                   # 65536 (A half / B half)
CH = 8192                         # moving columns per M-tile
NQ = HALF // CH                   # 8 M-tiles per core
REG = 2048                        # psum region columns
NREG = CH // REG                  # 4 regions per M-tile
S_FINE = 768.0
SS = 64.0 * S_FINE                # 49152
H_ROW = 2.0 ** 30
W_T = 0.3 / 3600.0
DC = 100.0                        # depth centering
TCEN = 1800.0                     # time centering
THRESH_K = 625.0 * S_FINE         # 480000

_CACHE = {}


def _build_module():
    import concourse.bacc as bacc
    import concourse.tile as tile
    from concourse import mybir

    f32 = mybir.dt.float32
    bf16 = mybir.dt.bfloat16
    i32 = mybir.dt.int32
    AF = mybir.ActivationFunctionType
    OP = mybir.AluOpType
    AX = mybir.AxisListType

    nc = bacc.Bacc("TRN2", target_bir_lowering=False)

    ddep = nc.dram_tensor("ddep", [N_C], f32, kind="ExternalInput")
    dtim = nc.dram_tensor("dtim", [N_C], f32, kind="ExternalInput")
    stat_in = nc.dram_tensor("stat", [40, 128], bf16, kind="ExternalInput")
    jb_in = nc.dram_tensor("jbias", [128, 1], f32, kind="ExternalInput")
    ones_in = nc.dram_tensor("ones4", [4, HALF], bf16, kind="ExternalInput")
    asn_out = nc.dram_tensor("asn", [64, 2048], i32, kind="ExternalOutput")
    wts_out = nc.dram_tensor("wts", [64, 2048], f32, kind="ExternalOutput")
    # bf16 moving-row scratch, columns = global detection index; 18 rows
    # (split-duplicated):
    #  0-5:  d'_0 d'_1 d'_2 d'_0 d'_1 d'_0   (coeff splits B0 B0 B0 B1 B1 B2)
    #  6-8:  d2_0 d2_1 d2_2                  (coeff SS exact)
    #  9-14: t_0 t_1 t_2 t_0 t_1 t_0         (coeff C0 C0 C0 C1 C1 C2)
    #  15-17: u_0 u_1 u_2                    (coeff SS exact)
    scratch = nc.dram_tensor("mscratch", [18, N_C], bf16)

    # scratch row lists per split index, offsets within a problem block
    DUP = {  # base -> {split -> [rows]}
        "d1": {0: [0, 3, 5], 1: [1, 4], 2: [2]},
        "d2": {0: [6], 1: [7], 2: [8]},
        "tau": {0: [9, 12, 14], 1: [10, 13], 2: [11]},
        "u": {0: [15], 1: [16], 2: [17]},
    }

    with tile.TileContext(nc) as tc:
        with (
            tc.tile_pool(name="const", bufs=1) as cpool,
            tc.tile_pool(name="prep", bufs=4) as prep,
            tc.tile_pool(name="load", bufs=2) as load,
            tc.tile_pool(name="mov", bufs=3) as mpool,
            tc.tile_pool(name="ps", bufs=2, space="PSUM") as ppool,
            tc.tile_pool(name="y", bufs=4) as ypool,
            tc.tile_pool(name="x", bufs=4) as xpool,
            tc.tile_pool(name="s1", bufs=1) as s1pool,
            tc.tile_pool(name="sh", bufs=2) as shpool,
            tc.tile_pool(name="post", bufs=2) as qpool,
        ):
            # ---- constants ----
            stat_t = cpool.tile([40, 128], bf16)
            nc.sync.dma_start(stat_t[:], stat_in[:])
            jb = cpool.tile([128, 1], f32)
            nc.sync.dma_start(jb[:], jb_in[:])
            bias_h = cpool.tile([128, 1], f32)
            nc.gpsimd.memset(bias_h[:], -(2.0 ** 30))
            bias_d = cpool.tile([128, 1], f32)
            nc.gpsimd.memset(bias_d[:], -DC)
            sw = float(np.float32(np.sqrt(W_T)))
            bias_t = cpool.tile([128, 1], f32)
            nc.gpsimd.memset(bias_t[:], -sw * TCEN)
            scale_t = cpool.tile([128, 1], f32)
            nc.gpsimd.memset(scale_t[:], sw)
            one_t = cpool.tile([128, 1], f32)
            nc.gpsimd.memset(one_t[:], 1.0)

            # ---- prep: f32 bases -> bf16 triple splits -> DRAM scratch ----
            dmaq = [nc.sync, nc.scalar]
            nd = [0]

            def wr(tile_, rows, hi):
                for r in rows:
                    eng = dmaq[nd[0] % 2]
                    nd[0] += 1
                    eng.dma_start(
                        scratch[r, hi * HALF:(hi + 1) * HALF].rearrange(
                            "(p f) -> p f", f=512),
                        tile_[:],
                    )

            for hi, h in enumerate(("A", "B")):
                off = 0 if h == "A" else HALF
                dload = load.tile([128, 512], f32, tag="dload")
                nc.sync.dma_start(
                    dload[:], ddep[off:off + HALF].rearrange("(p f) -> p f", f=512)
                )
                tload = load.tile([128, 512], f32, tag="tload")
                nc.scalar.dma_start(
                    tload[:], dtim[off:off + HALF].rearrange("(p f) -> p f", f=512)
                )
                bases = (
                    ("d1", dload, AF.Identity, one_t, bias_d),
                    ("d2", dload, AF.Square, one_t, bias_d),
                    ("tau", tload, AF.Identity, scale_t, bias_t),
                    ("u", tload, AF.Square, scale_t, bias_t),
                )
                for bn, srct, fn, sc, bias in bases:
                    x = prep.tile([128, 512], f32, tag="ppx")
                    nc.scalar.activation(x[:], srct[:], fn, bias=bias[:], scale=sc[:])
                    # triple split
                    s0 = prep.tile([128, 512], bf16, tag="pps0")
                    nc.vector.tensor_copy(s0[:], x[:])
                    r1 = prep.tile([128, 512], f32, tag="ppr1")
                    nc.gpsimd.tensor_tensor(out=r1[:], in0=x[:], in1=s0[:],
                                            op=OP.subtract)
                    s1t = prep.tile([128, 512], bf16, tag="pps1")
                    nc.vector.tensor_copy(s1t[:], r1[:])
                    r2 = prep.tile([128, 512], f32, tag="ppr2")
                    nc.vector.tensor_tensor(out=r2[:], in0=r1[:], in1=s1t[:],
                                            op=OP.subtract)
                    s2t = prep.tile([128, 512], bf16, tag="pps2")
                    nc.vector.tensor_copy(s2t[:], r2[:])
                    wr(s0, DUP[bn][0], hi)
                    wr(s1t, DUP[bn][1], hi)
                    wr(s2t, DUP[bn][2], hi)

            s1 = s1pool.tile([128, NQ * NREG * 64], f32)  # [128, 2048]

            # ---- main loop over M-tiles ----
            for q in range(NQ):
                m = mpool.tile([40, CH], bf16, tag="m")
                # rows 0-17: dets [16384q, +8192); rows 18-35: next 8192 dets
                nc.sync.dma_start(
                    m[0:18, :], scratch[:, 2 * q * CH:2 * q * CH + CH]
                )
                nc.sync.dma_start(
                    m[18:36, :], scratch[:, 2 * q * CH + CH:2 * (q + 1) * CH]
                )
                nc.scalar.dma_start(m[36:40, :], ones_in[:, q * CH:(q + 1) * CH])

                for r in range(NREG):
                    ps = ppool.tile([128, REG], f32, tag="ps")
                    for c in range(REG // 512):
                        col = r * REG + c * 512
                        nc.tensor.matmul(
                            ps[:, c * 512:(c + 1) * 512],
                            stat_t[:],
                            m[:, col:col + 512],
                            start=True,
                            stop=True,
                        )
                    y = ypool.tile([128, REG], f32, tag="y")
                    nc.scalar.activation(y[:], ps[:], AF.Identity, bias=bias_h[:])
                    x = xpool.tile([128, REG], f32, tag="x")
                    if r == 2:
                        nc.scalar.activation(x[:], y[:], AF.Identity, bias=jb[:])
                    elif q % 2 == 0 and r == 0:
                        nc.vector.tensor_scalar(
                            out=x[:], in0=y[:], scalar1=jb[:], scalar2=None,
                            op0=OP.add,
                        )
                    else:
                        nc.gpsimd.tensor_scalar(
                            out=x[:], in0=y[:], scalar1=jb[:], scalar2=None,
                            op0=OP.add,
                        )
                    nc.vector.tensor_reduce(
                        out=s1[:, q * 256 + r * 64:q * 256 + (r + 1) * 64],
                        in_=x[:].rearrange("p (b j) -> p b j", j=32),
                        op=OP.min,
                        axis=AX.X,
                        apply_transpose=True,
                    )

            # ---- stage 2 + post, incremental per 512-col chunk ----
            for ci in range(4):
                cs = slice(ci * 512, (ci + 1) * 512)
                shg1 = shpool.tile([64, 512], f32, tag="shg1")
                nc.sync.dma_start(shg1[0:32, :], s1[32:64, cs])
                nc.scalar.dma_start(shg1[32:64, :], s1[96:128, cs])
                shb0 = shpool.tile([64, 512], f32, tag="shb0")
                nc.sync.dma_start(shb0[32:64, :], s1[64:96, cs])
                fa = shpool.tile([64, 512], f32, tag="fa")
                nc.vector.tensor_tensor(
                    out=fa[0:32, :], in0=s1[0:32, cs], in1=shg1[0:32, :], op=OP.min
                )
                nc.vector.tensor_tensor(
                    out=fa[32:64, :], in0=shb0[32:64, :], in1=shg1[32:64, :],
                    op=OP.min,
                )
                ui = qpool.tile([64, 512], i32, tag="ui")
                nc.vector.tensor_copy(ui[:], fa[:])
                ji = qpool.tile([64, 512], i32, tag="ji")
                nc.vector.tensor_scalar(
                    out=ji[:], in0=ui[:], scalar1=63, scalar2=None, op0=OP.bitwise_and
                )
                nc.vector.tensor_scalar(
                    out=ui[:], in0=ui[:], scalar1=6, scalar2=None,
                    op0=OP.arith_shift_right,
                )
                kf = qpool.tile([64, 512], f32, tag="kf")
                nc.vector.tensor_copy(kf[:], ui[:])
                jf = qpool.tile([64, 512], f32, tag="jf")
                nc.gpsimd.tensor_copy(jf[:], ji[:])
                valid = qpool.tile([64, 512], f32, tag="valid")
                nc.gpsimd.tensor_scalar(
                    out=valid[:], in0=kf[:], scalar1=THRESH_K, scalar2=None,
                    op0=OP.is_lt,
                )
                # assignments = (jf + 1) * valid - 1
                nc.gpsimd.tensor_scalar(
                    out=jf[:], in0=jf[:], scalar1=1.0, scalar2=None, op0=OP.add
                )
                nc.gpsimd.tensor_tensor(out=jf[:], in0=jf[:], in1=valid[:],
                                        op=OP.mult)
                nc.gpsimd.tensor_scalar(
                    out=jf[:], in0=jf[:], scalar1=1.0, scalar2=None, op0=OP.subtract
                )
                nc.gpsimd.tensor_copy(ji[:], jf[:])
                nc.sync.dma_start(asn_out[:, cs], ji[:])
                # weights = valid / (1 + sqrt(kf/768))
                nc.vector.tensor_scalar(
                    out=kf[:], in0=kf[:], scalar1=1.0 / S_FINE, scalar2=None,
                    op0=OP.mult,
                )
                sq = qpool.tile([64, 512], f32, tag="sq")
                nc.scalar.activation(sq[:], kf[:], AF.Sqrt)
                nc.vector.tensor_scalar(
                    out=sq[:], in0=sq[:], scalar1=1.0, scalar2=None, op0=OP.add
                )
                nc.vector.reciprocal(sq[:], sq[:])
                nc.vector.tensor_tensor(out=sq[:], in0=sq[:], in1=valid[:],
                                        op=OP.mult)
                nc.scalar.dma_start(wts_out[:, cs], sq[:])

    nc.compile()
    return nc


def _host_consts(camera_depths, camera_times):
    import ml_dtypes
    bf = ml_dtypes.bfloat16

    def split3(x):
        x = np.asarray(x, np.float32)
        x0 = x.astype(bf).astype(np.float32)
        r1 = (x - x0).astype(np.float32)
        x1 = r1.astype(bf).astype(np.float32)
        r2 = (r1 - x1).astype(np.float32)
        x2 = r2.astype(bf).astype(np.float32)
        return x0, x1, x2

    cd = np.asarray(camera_depths, np.float64)
    ct = np.asarray(camera_times, np.float64)
    sw = float(np.float32(np.sqrt(W_T)))
    c1 = cd - DC
    t2c = sw * ct - sw * TCEN
    L = 0.5 * (1.0 - np.exp(-0.045 * cd))
    A = (SS * (c1 * c1 + t2c * t2c + L)).astype(np.float32)
    B = (SS * (-2.0 * c1)).astype(np.float32)
    C = (SS * (-2.0 * t2c)).astype(np.float32)
    Bs, Cs, As = split3(B), split3(C), split3(A)

    stat = np.zeros((40, 128), np.float32)
    jb = np.zeros((128, 1), np.float32)
    # per-problem data-row coeff layout (matches DUP in _build_module):
    # rows 0-5: B0 B0 B0 B1 B1 B2 ; 6-8: SS ; 9-14: C0 C0 C0 C1 C1 C2 ; 15-17: SS
    for g in range(4):
        cams = np.arange(32) + 32 * (g % 2)
        cols = slice(32 * g, 32 * g + 32)
        base = 0 if g < 2 else 18
        coefs = [Bs[0], Bs[0], Bs[0], Bs[1], Bs[1], Bs[2]]
        for ri, cf in enumerate(coefs):
            stat[base + ri, cols] = cf[cams]
        for ri in (6, 7, 8):
            stat[base + ri, cols] = SS
        coefs = [Cs[0], Cs[0], Cs[0], Cs[1], Cs[1], Cs[2]]
        for ri, cf in enumerate(coefs):
            stat[base + 9 + ri, cols] = cf[cams]
        for ri in (15, 16, 17):
            stat[base + ri, cols] = SS
        stat[36, cols] = As[0][cams]
        stat[37, cols] = As[1][cams]
        stat[38, cols] = As[2][cams]
        jb[cols, 0] = cams
    stat[39, :] = H_ROW
    # wait: rows 0-5 pattern above must pair with data splits d_0 d_1 d_2 d_0 d_1 d_0
    stat_b = stat.astype(bf)
    ones = np.ones((4, HALF), bf)
    return stat_b, jb, ones


def _det_perm():
    """device (p, m) -> core-local detection index, flattened [64*2048]."""
    p = np.arange(64)[:, None]
    m = np.arange(2048)[None, :]
    blk = p // 32
    i = p % 32
    q = m >> 8
    r = (m >> 6) & 3
    b = m & 63
    n = 2048 * r + 32 * b + i
    det = 2 * CH * q + CH * blk + n
    return det.ravel()


def kernel(detection_depths, camera_depths, detection_times, camera_times):
    from concourse.bass_utils import run_bass_kernel_spmd

    if "nc" not in _CACHE:
        _CACHE["nc"] = _build_module()
        _CACHE["perm"] = _det_perm()
    nc = _CACHE["nc"]
    perm = _CACHE["perm"]

    dd = np.ascontiguousarray(np.asarray(detection_depths, np.float32))
    dt = np.ascontiguousarray(np.asarray(detection_times, np.float32))
    stat, jb, ones = _host_consts(camera_depths, camera_times)

    in_maps = []
    for c in range(N_CORES):
        sl = slice(c * N_C, (c + 1) * N_C)
        in_maps.append({
            "ddep": dd[sl].copy(),
            "dtim": dt[sl].copy(),
            "stat": stat,
            "jbias": jb,
            "ones4": ones,
        })
    results = run_bass_kernel_spmd(nc, in_maps, list(range(N_CORES))).results

    assignments = np.empty(N_TOTAL, np.int32)
    weights = np.empty(N_TOTAL, np.float32)
    for c in range(N_CORES):
        base = c * N_C
        a_loc = np.empty(N_C, np.int32)
        w_loc = np.empty(N_C, np.float32)
        a_loc[perm] = results[c]["asn"].ravel()
        w_loc[perm] = results[c]["wts"].ravel()
        assignments[base:base + N_C] = a_loc
        weights[base:base + N_C] = w_loc
    return assignments, weights


# revision 23
# speedup vs baseline: 811.8483x; 1.0137x over previous
"""DepthWeightedAssignment Trainium2 kernel.

Per-row (detection) argmin over 64 cameras of
  cost[i,j] = (d_i-c_j)^2 + 0.5*(1-exp(-0.045 c_j)) + 0.3*(t_i-t_j)^2/3600
plus threshold/weight postprocessing, sharded over 8 NeuronCores (N axis).

Device algorithm (per core, N_c = 131072 detections):
  - PE matmul computes V = fl(49152*cost + 2^30) with the 2^30 row
    accumulated last, so V - 2^30 = 64*k exactly, k = round(768*cost).
    Layout: PSUM [128 partitions = (A cams0-31 | A cams32-63 | B cams0-31 |
    B cams32-63), free = detection columns]; two detections (A,B) share each
    moving column via disjoint K-rows.
  - ACT subtracts 2^30 (exact power-of-two bias) -> Y = 64*k.
  - GPSIMD adds per-partition camera index j -> X = 64*k + j (exact fp32
    integers for any row-minimum; losers stay strictly larger).
  - DVE tensor_reduce(min, axis=X, apply_transpose=True) reduces each
    32-camera partition group into the free dim (32x32 stream transpose),
    all 128 lanes active.
  - Small TT-min combines the two 32-cam groups; int unpack j = X & 63,
    k = X >> 6; weights = valid/(1+sqrt(k/768)).
Host side only shards inputs, builds O(64) camera coefficient tables, and
un-permutes the outputs.
"""
import sys

sys.path.insert(0, "/opt/trn_rl_repo")

import numpy as np

N_TOTAL = 1 << 20
M_CAMS = 64
N_CORES = 8
N_C = N_TOTAL // N_CORES          # 131072 per core
HALF = N_C // 2                   # 65536 (A half / B half)
CH = 8192                         # moving columns per M-tile
NQ = HALF // CH                   # 8 M-tiles per core
REG = 2048                        # psum region columns
NREG = CH // REG                  # 4 regions per M-tile
S_FINE = 768.0
SS = 64.0 * S_FINE                # 49152
H_ROW = 2.0 ** 30
W_T = 0.3 / 3600.0
DC = 100.0                        # depth centering
TCEN = 1800.0                     # time centering
THRESH_K = 625.0 * S_FINE         # 480000

_CACHE = {}


def _build_module():
    import concourse.bacc as bacc
    import concourse.tile as tile
    from concourse import mybir

    f32 = mybir.dt.float32
    bf16 = mybir.dt.bfloat16
    i32 = mybir.dt.int32
    AF = mybir.ActivationFunctionType
    OP = mybir.AluOpType
    AX = mybir.AxisListType

    nc = bacc.Bacc("TRN2", target_bir_lowering=False)

    ddep = nc.dram_tensor("ddep", [N_C], f32, kind="ExternalInput")
    dtim = nc.dram_tensor("dtim", [N_C], f32, kind="ExternalInput")
    stat_in = nc.dram_tensor("stat", [40, 128], bf16, kind="ExternalInput")
    jb_in = nc.dram_tensor("jbias", [128, 1], f32, kind="ExternalInput")
    ones_in = nc.dram_tensor("ones4", [4, HALF], bf16, kind="ExternalInput")
    asn_out = nc.dram_tensor("asn", [64, 2048], i32, kind="ExternalOutput")
    wts_out = nc.dram_tensor("wts", [64, 2048], f32, kind="ExternalOutput")
    # bf16 moving-row scratch, columns = global detection index; 18 rows
    # (split-duplicated):
    #  0-5:  d'_0 d'_1 d'_2 d'_0 d'_1 d'_0   (coeff splits B0 B0 B0 B1 B1 B2)
    #  6-8:  d2_0 d2_1 d2_2                  (coeff SS exact)
    #  9-14: t_0 t_1 t_2 t_0 t_1 t_0         (coeff C0 C0 C0 C1 C1 C2)
    #  15-17: u_0 u_1 u_2                    (coeff SS exact)
    scratch = nc.dram_tensor("mscratch", [18, N_C], bf16)

    # scratch row lists per split index, offsets within a problem block
    DUP = {  # base -> {split -> [rows]}
        "d1": {0: [0, 3, 5], 1: [1, 4], 2: [2]},
        "d2": {0: [6], 1: [7], 2: [8]},
        "tau": {0: [9, 12, 14], 1: [10, 13], 2: [11]},
        "u": {0: [15], 1: [16], 2: [17]},
    }

    with tile.TileContext(nc) as tc:
        with (
            tc.tile_pool(name="const", bufs=1) as cpool,
            tc.tile_pool(name="prep", bufs=4) as prep,
            tc.tile_pool(name="load", bufs=2) as load,
            tc.tile_pool(name="mov", bufs=3) as mpool,
            tc.tile_pool(name="ps", bufs=2, space="PSUM") as ppool,
            tc.tile_pool(name="y", bufs=4) as ypool,
            tc.tile_pool(name="x", bufs=4) as xpool,
            tc.tile_pool(name="s1", bufs=1) as s1pool,
            tc.tile_pool(name="sh", bufs=2) as shpool,
            tc.tile_pool(name="post", bufs=2) as qpool,
        ):
            # ---- constants ----
            stat_t = cpool.tile([40, 128], bf16)
            nc.sync.dma_start(stat_t[:], stat_in[:])
            jb = cpool.tile([128, 1], f32)
            nc.sync.dma_start(jb[:], jb_in[:])
            bias_h = cpool.tile([128, 1], f32)
            nc.gpsimd.memset(bias_h[:], -(2.0 ** 30))
            bias_d = cpool.tile([128, 1], f32)
            nc.gpsimd.memset(bias_d[:], -DC)
            sw = float(np.float32(np.sqrt(W_T)))
            bias_t = cpool.tile([128, 1], f32)
            nc.gpsimd.memset(bias_t[:], -sw * TCEN)
            scale_t = cpool.tile([128, 1], f32)
            nc.gpsimd.memset(scale_t[:], sw)
            one_t = cpool.tile([128, 1], f32)
            nc.gpsimd.memset(one_t[:], 1.0)

            # ---- prep: f32 bases -> bf16 triple splits -> DRAM scratch ----
            def wr(tile_, rows, hi):
                for r in rows:
                    nc.sync.dma_start(
                        scratch[r, hi * HALF:(hi + 1) * HALF].rearrange(
                            "(p f) -> p f", f=512),
                        tile_[:],
                    )

            for hi, h in enumerate(("A", "B")):
                off = 0 if h == "A" else HALF
                dload = load.tile([128, 512], f32, tag="dload")
                nc.sync.dma_start(
                    dload[:], ddep[off:off + HALF].rearrange("(p f) -> p f", f=512)
                )
                tload = load.tile([128, 512], f32, tag="tload")
                nc.scalar.dma_start(
                    tload[:], dtim[off:off + HALF].rearrange("(p f) -> p f", f=512)
                )
                bases = (
                    ("d1", dload, AF.Identity, one_t, bias_d),
                    ("d2", dload, AF.Square, one_t, bias_d),
                    ("tau", tload, AF.Identity, scale_t, bias_t),
                    ("u", tload, AF.Square, scale_t, bias_t),
                )
                for bn, srct, fn, sc, bias in bases:
                    x = prep.tile([128, 512], f32, tag="ppx")
                    nc.scalar.activation(x[:], srct[:], fn, bias=bias[:], scale=sc[:])
                    # triple split
                    s0 = prep.tile([128, 512], bf16, tag="pps0")
                    nc.vector.tensor_copy(s0[:], x[:])
                    r1 = prep.tile([128, 512], f32, tag="ppr1")
                    nc.gpsimd.tensor_tensor(out=r1[:], in0=x[:], in1=s0[:],
                                            op=OP.subtract)
                    s1t = prep.tile([128, 512], bf16, tag="pps1")
                    nc.vector.tensor_copy(s1t[:], r1[:])
                    r2 = prep.tile([128, 512], f32, tag="ppr2")
                    nc.vector.tensor_tensor(out=r2[:], in0=r1[:], in1=s1t[:],
                                            op=OP.subtract)
                    s2t = prep.tile([128, 512], bf16, tag="pps2")
                    nc.vector.tensor_copy(s2t[:], r2[:])
                    wr(s0, DUP[bn][0], hi)
                    wr(s1t, DUP[bn][1], hi)
                    wr(s2t, DUP[bn][2], hi)

            s1 = s1pool.tile([128, NQ * NREG * 64], f32)  # [128, 2048]

            # ---- main loop over M-tiles ----
            for q in range(NQ):
                m = mpool.tile([40, CH], bf16, tag="m")
                # rows 0-17: dets [16384q, +8192); rows 18-35: next 8192 dets
                nc.sync.dma_start(
                    m[0:18, :], scratch[:, 2 * q * CH:2 * q * CH + CH]
                )
                nc.sync.dma_start(
                    m[18:36, :], scratch[:, 2 * q * CH + CH:2 * (q + 1) * CH]
                )
                nc.scalar.dma_start(m[36:40, :], ones_in[:, q * CH:(q + 1) * CH])

                for r in range(NREG):
                    ps = ppool.tile([128, REG], f32, tag="ps")
                    for c in range(REG // 512):
                        col = r * REG + c * 512
                        nc.tensor.matmul(
                            ps[:, c * 512:(c + 1) * 512],
                            stat_t[:],
                            m[:, col:col + 512],
                            start=True,
                            stop=True,
                        )
                    y = ypool.tile([128, REG], f32, tag="y")
                    nc.scalar.activation(y[:], ps[:], AF.Identity, bias=bias_h[:])
                    x = xpool.tile([128, REG], f32, tag="x")
                    if r == 2:
                        nc.scalar.activation(x[:], y[:], AF.Identity, bias=jb[:])
                    elif q % 2 == 0 and r == 0:
                        nc.vector.tensor_scalar(
                            out=x[:], in0=y[:], scalar1=jb[:], scalar2=None,
                            op0=OP.add,
                        )
                    else:
                        nc.gpsimd.tensor_scalar(
                            out=x[:], in0=y[:], scalar1=jb[:], scalar2=None,
                            op0=OP.add,
                        )
                    nc.vector.tensor_reduce(
                        out=s1[:, q * 256 + r * 64:q * 256 + (r + 1) * 64],
                        in_=x[:].rearrange("p (b j) -> p b j", j=32),
                        op=OP.min,
                        axis=AX.X,
                        apply_transpose=True,
                    )

            # ---- stage 2 + post, incremental per 512-col chunk ----
            for ci in range(4):
                cs = slice(ci * 512, (ci + 1) * 512)
                shg1 = shpool.tile([64, 512], f32, tag="shg1")
                nc.sync.dma_start(shg1[0:32, :], s1[32:64, cs])
                nc.sync.dma_start(shg1[32:64, :], s1[96:128, cs])
                shb0 = shpool.tile([64, 512], f32, tag="shb0")
                nc.sync.dma_start(shb0[32:64, :], s1[64:96, cs])
                fa = shpool.tile([64, 512], f32, tag="fa")
                nc.vector.tensor_tensor(
                    out=fa[0:32, :], in0=s1[0:32, cs], in1=shg1[0:32, :], op=OP.min
                )
                nc.vector.tensor_tensor(
                    out=fa[32:64, :], in0=shb0[32:64, :], in1=shg1[32:64, :],
                    op=OP.min,
                )
                ui = qpool.tile([64, 512], i32, tag="ui")
                nc.vector.tensor_copy(ui[:], fa[:])
                ji = qpool.tile([64, 512], i32, tag="ji")
                nc.vector.tensor_scalar(
                    out=ji[:], in0=ui[:], scalar1=63, scalar2=None, op0=OP.bitwise_and
                )
                nc.vector.tensor_scalar(
                    out=ui[:], in0=ui[:], scalar1=6, scalar2=None,
                    op0=OP.arith_shift_right,
                )
                kf = qpool.tile([64, 512], f32, tag="kf")
                nc.vector.tensor_copy(kf[:], ui[:])
                jf = qpool.tile([64, 512], f32, tag="jf")
                nc.gpsimd.tensor_copy(jf[:], ji[:])
                valid = qpool.tile([64, 512], f32, tag="valid")
                nc.gpsimd.tensor_scalar(
                    out=valid[:], in0=kf[:], scalar1=THRESH_K, scalar2=None,
                    op0=OP.is_lt,
                )
                # assignments = (jf + 1) * valid - 1
                nc.gpsimd.tensor_scalar(
                    out=jf[:], in0=jf[:], scalar1=1.0, scalar2=None, op0=OP.add
                )
                nc.gpsimd.tensor_tensor(out=jf[:], in0=jf[:], in1=valid[:],
                                        op=OP.mult)
                nc.gpsimd.tensor_scalar(
                    out=jf[:], in0=jf[:], scalar1=1.0, scalar2=None, op0=OP.subtract
                )
                nc.gpsimd.tensor_copy(ji[:], jf[:])
                nc.sync.dma_start(asn_out[:, cs], ji[:])
                # weights = valid / (1 + sqrt(kf/768))
                nc.vector.tensor_scalar(
                    out=kf[:], in0=kf[:], scalar1=1.0 / S_FINE, scalar2=None,
                    op0=OP.mult,
                )
                sq = qpool.tile([64, 512], f32, tag="sq")
                nc.scalar.activation(sq[:], kf[:], AF.Sqrt)
                nc.vector.tensor_scalar(
                    out=sq[:], in0=sq[:], scalar1=1.0, scalar2=None, op0=OP.add
                )
                nc.vector.reciprocal(sq[:], sq[:])
                nc.vector.tensor_tensor(out=sq[:], in0=sq[:], in1=valid[:],
                                        op=OP.mult)
                nc.sync.dma_start(wts_out[:, cs], sq[:])

    nc.compile()
    return nc


def _host_consts(camera_depths, camera_times):
    import ml_dtypes
    bf = ml_dtypes.bfloat16

    def split3(x):
        x = np.asarray(x, np.float32)
        x0 = x.astype(bf).astype(np.float32)
        r1 = (x - x0).astype(np.float32)
        x1 = r1.astype(bf).astype(np.float32)
        r2 = (r1 - x1).astype(np.float32)
        x2 = r2.astype(bf).astype(np.float32)
        return x0, x1, x2

    cd = np.asarray(camera_depths, np.float64)
    ct = np.asarray(camera_times, np.float64)
    sw = float(np.float32(np.sqrt(W_T)))
    c1 = cd - DC
    t2c = sw * ct - sw * TCEN
    L = 0.5 * (1.0 - np.exp(-0.045 * cd))
    A = (SS * (c1 * c1 + t2c * t2c + L)).astype(np.float32)
    B = (SS * (-2.0 * c1)).astype(np.float32)
    C = (SS * (-2.0 * t2c)).astype(np.float32)
    Bs, Cs, As = split3(B), split3(C), split3(A)

    stat = np.zeros((40, 128), np.float32)
    jb = np.zeros((128, 1), np.float32)
    # per-problem data-row coeff layout (matches DUP in _build_module):
    # rows 0-5: B0 B0 B0 B1 B1 B2 ; 6-8: SS ; 9-14: C0 C0 C0 C1 C1 C2 ; 15-17: SS
    for g in range(4):
        cams = np.arange(32) + 32 * (g % 2)
        cols = slice(32 * g, 32 * g + 32)
        base = 0 if g < 2 else 18
        coefs = [Bs[0], Bs[0], Bs[0], Bs[1], Bs[1], Bs[2]]
        for ri, cf in enumerate(coefs):
            stat[base + ri, cols] = cf[cams]
        for ri in (6, 7, 8):
            stat[base + ri, cols] = SS
        coefs = [Cs[0], Cs[0], Cs[0], Cs[1], Cs[1], Cs[2]]
        for ri, cf in enumerate(coefs):
            stat[base + 9 + ri, cols] = cf[cams]
        for ri in (15, 16, 17):
            stat[base + ri, cols] = SS
        stat[36, cols] = As[0][cams]
        stat[37, cols] = As[1][cams]
        stat[38, cols] = As[2][cams]
        jb[cols, 0] = cams
    stat[39, :] = H_ROW
    # wait: rows 0-5 pattern above must pair with data splits d_0 d_1 d_2 d_0 d_1 d_0
    stat_b = stat.astype(bf)
    ones = np.ones((4, HALF), bf)
    return stat_b, jb, ones


def _det_perm():
    """device (p, m) -> core-local detection index, flattened [64*2048]."""
    p = np.arange(64)[:, None]
    m = np.arange(2048)[None, :]
    blk = p // 32
    i = p % 32
    q = m >> 8
    r = (m >> 6) & 3
    b = m & 63
    n = 2048 * r + 32 * b + i
    det = 2 * CH * q + CH * blk + n
    return det.ravel()


def kernel(detection_depths, camera_depths, detection_times, camera_times):
    from concourse.bass_utils import run_bass_kernel_spmd

    if "nc" not in _CACHE:
        _CACHE["nc"] = _build_module()
        _CACHE["perm"] = _det_perm()
    nc = _CACHE["nc"]
    perm = _CACHE["perm"]

    dd = np.ascontiguousarray(np.asarray(detection_depths, np.float32))
    dt = np.ascontiguousarray(np.asarray(detection_times, np.float32))
    stat, jb, ones = _host_consts(camera_depths, camera_times)

    in_maps = []
    for c in range(N_CORES):
        sl = slice(c * N_C, (c + 1) * N_C)
        in_maps.append({
            "ddep": dd[sl].copy(),
            "dtim": dt[sl].copy(),
            "stat": stat,
            "jbias": jb,
            "ones4": ones,
        })
    results = run_bass_kernel_spmd(nc, in_maps, list(range(N_CORES))).results

    assignments = np.empty(N_TOTAL, np.int32)
    weights = np.empty(N_TOTAL, np.float32)
    for c in range(N_CORES):
        base = c * N_C
        a_loc = np.empty(N_C, np.int32)
        w_loc = np.empty(N_C, np.float32)
        a_loc[perm] = results[c]["asn"].ravel()
        w_loc[perm] = results[c]["wts"].ravel()
        assignments[base:base + N_C] = a_loc
        weights[base:base + N_C] = w_loc
    return assignments, weights


# revision 24
# speedup vs baseline: 826.9832x; 1.0186x over previous
"""DepthWeightedAssignment Trainium2 kernel.

Per-row (detection) argmin over 64 cameras of
  cost[i,j] = (d_i-c_j)^2 + 0.5*(1-exp(-0.045 c_j)) + 0.3*(t_i-t_j)^2/3600
plus threshold/weight postprocessing, sharded over 8 NeuronCores (N axis).

Device algorithm (per core, N_c = 131072 detections):
  - PE matmul computes V = fl(49152*cost + 2^30) with the 2^30 row
    accumulated last, so V - 2^30 = 64*k exactly, k = round(768*cost).
    Layout: PSUM [128 partitions = (A cams0-31 | A cams32-63 | B cams0-31 |
    B cams32-63), free = detection columns]; two detections (A,B) share each
    moving column via disjoint K-rows.
  - ACT subtracts 2^30 (exact power-of-two bias) -> Y = 64*k.
  - GPSIMD adds per-partition camera index j -> X = 64*k + j (exact fp32
    integers for any row-minimum; losers stay strictly larger).
  - DVE tensor_reduce(min, axis=X, apply_transpose=True) reduces each
    32-camera partition group into the free dim (32x32 stream transpose),
    all 128 lanes active.
  - Small TT-min combines the two 32-cam groups; int unpack j = X & 63,
    k = X >> 6; weights = valid/(1+sqrt(k/768)).
Host side only shards inputs, builds O(64) camera coefficient tables, and
un-permutes the outputs.
"""
import sys

sys.path.insert(0, "/opt/trn_rl_repo")

import numpy as np

N_TOTAL = 1 << 20
M_CAMS = 64
N_CORES = 8
N_C = N_TOTAL // N_CORES          # 131072 per core
HALF = N_C // 2                   # 65536 (A half / B half)
CH = 8192                         # moving columns per M-tile
NQ = HALF // CH                   # 8 M-tiles per core
REG = 2048                        # psum region columns
NREG = CH // REG                  # 4 regions per M-tile
S_FINE = 768.0
SS = 64.0 * S_FINE                # 49152
H_ROW = 2.0 ** 30
W_T = 0.3 / 3600.0
DC = 100.0                        # depth centering
TCEN = 1800.0                     # time centering
THRESH_K = 625.0 * S_FINE         # 480000

_CACHE = {}


def _build_module():
    import concourse.bacc as bacc
    import concourse.tile as tile
    from concourse import mybir

    f32 = mybir.dt.float32
    bf16 = mybir.dt.bfloat16
    i32 = mybir.dt.int32
    AF = mybir.ActivationFunctionType
    OP = mybir.AluOpType
    AX = mybir.AxisListType

    nc = bacc.Bacc("TRN2", target_bir_lowering=False)

    ddep = nc.dram_tensor("ddep", [N_C], f32, kind="ExternalInput")
    dtim = nc.dram_tensor("dtim", [N_C], f32, kind="ExternalInput")
    stat_in = nc.dram_tensor("stat", [40, 128], bf16, kind="ExternalInput")
    jb_in = nc.dram_tensor("jbias", [128, 1], f32, kind="ExternalInput")
    ones_in = nc.dram_tensor("ones4", [4, HALF], bf16, kind="ExternalInput")
    asn_out = nc.dram_tensor("asn", [64, 2048], i32, kind="ExternalOutput")
    wts_out = nc.dram_tensor("wts", [64, 2048], f32, kind="ExternalOutput")
    # bf16 moving-row scratch, columns = global detection index; 18 rows
    # (split-duplicated):
    #  0-5:  d'_0 d'_1 d'_2 d'_0 d'_1 d'_0   (coeff splits B0 B0 B0 B1 B1 B2)
    #  6-8:  d2_0 d2_1 d2_2                  (coeff SS exact)
    #  9-14: t_0 t_1 t_2 t_0 t_1 t_0         (coeff C0 C0 C0 C1 C1 C2)
    #  15-17: u_0 u_1 u_2                    (coeff SS exact)
    scratch = nc.dram_tensor("mscratch", [18, N_C], bf16)

    # scratch row lists per split index, offsets within a problem block
    DUP = {  # base -> {split -> [rows]}
        "d1": {0: [0, 3, 5], 1: [1, 4], 2: [2]},
        "d2": {0: [6], 1: [7], 2: [8]},
        "tau": {0: [9, 12, 14], 1: [10, 13], 2: [11]},
        "u": {0: [15], 1: [16], 2: [17]},
    }

    with tile.TileContext(nc) as tc:
        with (
            tc.tile_pool(name="const", bufs=1) as cpool,
            tc.tile_pool(name="prep", bufs=4) as prep,
            tc.tile_pool(name="load", bufs=2) as load,
            tc.tile_pool(name="mov", bufs=3) as mpool,
            tc.tile_pool(name="ps", bufs=2, space="PSUM") as ppool,
            tc.tile_pool(name="y", bufs=4) as ypool,
            tc.tile_pool(name="x", bufs=4) as xpool,
            tc.tile_pool(name="s1", bufs=1) as s1pool,
            tc.tile_pool(name="sh", bufs=2) as shpool,
            tc.tile_pool(name="post", bufs=2) as qpool,
        ):
            # ---- constants ----
            stat_t = cpool.tile([40, 128], bf16)
            nc.sync.dma_start(stat_t[:], stat_in[:])
            jb = cpool.tile([128, 1], f32)
            nc.sync.dma_start(jb[:], jb_in[:])
            bias_h = cpool.tile([128, 1], f32)
            nc.gpsimd.memset(bias_h[:], -(2.0 ** 30))
            bias_d = cpool.tile([128, 1], f32)
            nc.gpsimd.memset(bias_d[:], -DC)
            sw = float(np.float32(np.sqrt(W_T)))
            bias_t = cpool.tile([128, 1], f32)
            nc.gpsimd.memset(bias_t[:], -sw * TCEN)
            scale_t = cpool.tile([128, 1], f32)
            nc.gpsimd.memset(scale_t[:], sw)
            one_t = cpool.tile([128, 1], f32)
            nc.gpsimd.memset(one_t[:], 1.0)

            # ---- prep: f32 bases -> bf16 triple splits -> DRAM scratch ----
            def wr(tile_, rows, hi):
                for r in rows:
                    nc.sync.dma_start(
                        scratch[r, hi * HALF:(hi + 1) * HALF].rearrange(
                            "(p f) -> p f", f=512),
                        tile_[:],
                    )

            def do_prep(hi):
                off = hi * HALF
                dload = load.tile([128, 512], f32, tag="dload")
                nc.sync.dma_start(
                    dload[:], ddep[off:off + HALF].rearrange("(p f) -> p f", f=512)
                )
                tload = load.tile([128, 512], f32, tag="tload")
                nc.scalar.dma_start(
                    tload[:], dtim[off:off + HALF].rearrange("(p f) -> p f", f=512)
                )
                bases = (
                    ("d1", dload, AF.Identity, one_t, bias_d),
                    ("d2", dload, AF.Square, one_t, bias_d),
                    ("tau", tload, AF.Identity, scale_t, bias_t),
                    ("u", tload, AF.Square, scale_t, bias_t),
                )
                for bn, srct, fn, sc, bias in bases:
                    x = prep.tile([128, 512], f32, tag="ppx")
                    nc.scalar.activation(x[:], srct[:], fn, bias=bias[:], scale=sc[:])
                    # triple split
                    s0 = prep.tile([128, 512], bf16, tag="pps0")
                    nc.vector.tensor_copy(s0[:], x[:])
                    r1 = prep.tile([128, 512], f32, tag="ppr1")
                    nc.gpsimd.tensor_tensor(out=r1[:], in0=x[:], in1=s0[:],
                                            op=OP.subtract)
                    s1t = prep.tile([128, 512], bf16, tag="pps1")
                    nc.vector.tensor_copy(s1t[:], r1[:])
                    r2 = prep.tile([128, 512], f32, tag="ppr2")
                    nc.vector.tensor_tensor(out=r2[:], in0=r1[:], in1=s1t[:],
                                            op=OP.subtract)
                    s2t = prep.tile([128, 512], bf16, tag="pps2")
                    nc.vector.tensor_copy(s2t[:], r2[:])
                    wr(s0, DUP[bn][0], hi)
                    wr(s1t, DUP[bn][1], hi)
                    wr(s2t, DUP[bn][2], hi)

            s1 = s1pool.tile([128, NQ * NREG * 64], f32)  # [128, 2048]

            # A-half prep first; B-half emitted after q=0 so the first
            # M-fills (q 0-3 read only A-half columns) aren't stuck behind
            # blocked B scratch-writes in the SP dispatch FIFO.
            do_prep(0)

            # ---- main loop over M-tiles ----
            for q in range(NQ):
                if q == 1:
                    do_prep(1)
                m = mpool.tile([40, CH], bf16, tag="m")
                # rows 0-17: dets [16384q, +8192); rows 18-35: next 8192 dets
                nc.sync.dma_start(
                    m[0:18, :], scratch[:, 2 * q * CH:2 * q * CH + CH]
                )
                nc.sync.dma_start(
                    m[18:36, :], scratch[:, 2 * q * CH + CH:2 * (q + 1) * CH]
                )
                nc.scalar.dma_start(m[36:40, :], ones_in[:, q * CH:(q + 1) * CH])

                for r in range(NREG):
                    ps = ppool.tile([128, REG], f32, tag="ps")
                    for c in range(REG // 512):
                        col = r * REG + c * 512
                        nc.tensor.matmul(
                            ps[:, c * 512:(c + 1) * 512],
                            stat_t[:],
                            m[:, col:col + 512],
                            start=True,
                            stop=True,
                        )
                    y = ypool.tile([128, REG], f32, tag="y")
                    nc.scalar.activation(y[:], ps[:], AF.Identity, bias=bias_h[:])
                    x = xpool.tile([128, REG], f32, tag="x")
                    if r == 2:
                        nc.scalar.activation(x[:], y[:], AF.Identity, bias=jb[:])
                    elif q % 2 == 0 and r == 0:
                        nc.vector.tensor_scalar(
                            out=x[:], in0=y[:], scalar1=jb[:], scalar2=None,
                            op0=OP.add,
                        )
                    else:
                        nc.gpsimd.tensor_scalar(
                            out=x[:], in0=y[:], scalar1=jb[:], scalar2=None,
                            op0=OP.add,
                        )
                    nc.vector.tensor_reduce(
                        out=s1[:, q * 256 + r * 64:q * 256 + (r + 1) * 64],
                        in_=x[:].rearrange("p (b j) -> p b j", j=32),
                        op=OP.min,
                        axis=AX.X,
                        apply_transpose=True,
                    )

            # ---- stage 2 + post, incremental per 512-col chunk ----
            for ci in range(4):
                cs = slice(ci * 512, (ci + 1) * 512)
                shg1 = shpool.tile([64, 512], f32, tag="shg1")
                nc.sync.dma_start(shg1[0:32, :], s1[32:64, cs])
                nc.sync.dma_start(shg1[32:64, :], s1[96:128, cs])
                shb0 = shpool.tile([64, 512], f32, tag="shb0")
                nc.sync.dma_start(shb0[32:64, :], s1[64:96, cs])
                fa = shpool.tile([64, 512], f32, tag="fa")
                nc.vector.tensor_tensor(
                    out=fa[0:32, :], in0=s1[0:32, cs], in1=shg1[0:32, :], op=OP.min
                )
                nc.vector.tensor_tensor(
                    out=fa[32:64, :], in0=shb0[32:64, :], in1=shg1[32:64, :],
                    op=OP.min,
                )
                ui = qpool.tile([64, 512], i32, tag="ui")
                nc.vector.tensor_copy(ui[:], fa[:])
                ji = qpool.tile([64, 512], i32, tag="ji")
                nc.vector.tensor_scalar(
                    out=ji[:], in0=ui[:], scalar1=63, scalar2=None, op0=OP.bitwise_and
                )
                nc.vector.tensor_scalar(
                    out=ui[:], in0=ui[:], scalar1=6, scalar2=None,
                    op0=OP.arith_shift_right,
                )
                kf = qpool.tile([64, 512], f32, tag="kf")
                nc.vector.tensor_copy(kf[:], ui[:])
                jf = qpool.tile([64, 512], f32, tag="jf")
                nc.gpsimd.tensor_copy(jf[:], ji[:])
                valid = qpool.tile([64, 512], f32, tag="valid")
                nc.gpsimd.tensor_scalar(
                    out=valid[:], in0=kf[:], scalar1=THRESH_K, scalar2=None,
                    op0=OP.is_lt,
                )
                # assignments = (jf + 1) * valid - 1
                nc.gpsimd.tensor_scalar(
                    out=jf[:], in0=jf[:], scalar1=1.0, scalar2=None, op0=OP.add
                )
                nc.gpsimd.tensor_tensor(out=jf[:], in0=jf[:], in1=valid[:],
                                        op=OP.mult)
                nc.gpsimd.tensor_scalar(
                    out=jf[:], in0=jf[:], scalar1=1.0, scalar2=None, op0=OP.subtract
                )
                nc.gpsimd.tensor_copy(ji[:], jf[:])
                nc.sync.dma_start(asn_out[:, cs], ji[:])
                # weights = valid / (1 + sqrt(kf/768))
                nc.vector.tensor_scalar(
                    out=kf[:], in0=kf[:], scalar1=1.0 / S_FINE, scalar2=None,
                    op0=OP.mult,
                )
                sq = qpool.tile([64, 512], f32, tag="sq")
                nc.scalar.activation(sq[:], kf[:], AF.Sqrt)
                nc.vector.tensor_scalar(
                    out=sq[:], in0=sq[:], scalar1=1.0, scalar2=None, op0=OP.add
                )
                nc.vector.reciprocal(sq[:], sq[:])
                nc.vector.tensor_tensor(out=sq[:], in0=sq[:], in1=valid[:],
                                        op=OP.mult)
                nc.sync.dma_start(wts_out[:, cs], sq[:])

    nc.compile()
    return nc


def _host_consts(camera_depths, camera_times):
    import ml_dtypes
    bf = ml_dtypes.bfloat16

    def split3(x):
        x = np.asarray(x, np.float32)
        x0 = x.astype(bf).astype(np.float32)
        r1 = (x - x0).astype(np.float32)
        x1 = r1.astype(bf).astype(np.float32)
        r2 = (r1 - x1).astype(np.float32)
        x2 = r2.astype(bf).astype(np.float32)
        return x0, x1, x2

    cd = np.asarray(camera_depths, np.float64)
    ct = np.asarray(camera_times, np.float64)
    sw = float(np.float32(np.sqrt(W_T)))
    c1 = cd - DC
    t2c = sw * ct - sw * TCEN
    L = 0.5 * (1.0 - np.exp(-0.045 * cd))
    A = (SS * (c1 * c1 + t2c * t2c + L)).astype(np.float32)
    B = (SS * (-2.0 * c1)).astype(np.float32)
    C = (SS * (-2.0 * t2c)).astype(np.float32)
    Bs, Cs, As = split3(B), split3(C), split3(A)

    stat = np.zeros((40, 128), np.float32)
    jb = np.zeros((128, 1), np.float32)
    # per-problem data-row coeff layout (matches DUP in _build_module):
    # rows 0-5: B0 B0 B0 B1 B1 B2 ; 6-8: SS ; 9-14: C0 C0 C0 C1 C1 C2 ; 15-17: SS
    for g in range(4):
        cams = np.arange(32) + 32 * (g % 2)
        cols = slice(32 * g, 32 * g + 32)
        base = 0 if g < 2 else 18
        coefs = [Bs[0], Bs[0], Bs[0], Bs[1], Bs[1], Bs[2]]
        for ri, cf in enumerate(coefs):
            stat[base + ri, cols] = cf[cams]
        for ri in (6, 7, 8):
            stat[base + ri, cols] = SS
        coefs = [Cs[0], Cs[0], Cs[0], Cs[1], Cs[1], Cs[2]]
        for ri, cf in enumerate(coefs):
            stat[base + 9 + ri, cols] = cf[cams]
        for ri in (15, 16, 17):
            stat[base + ri, cols] = SS
        stat[36, cols] = As[0][cams]
        stat[37, cols] = As[1][cams]
        stat[38, cols] = As[2][cams]
        jb[cols, 0] = cams
    stat[39, :] = H_ROW
    # wait: rows 0-5 pattern above must pair with data splits d_0 d_1 d_2 d_0 d_1 d_0
    stat_b = stat.astype(bf)
    ones = np.ones((4, HALF), bf)
    return stat_b, jb, ones


def _det_perm():
    """device (p, m) -> core-local detection index, flattened [64*2048]."""
    p = np.arange(64)[:, None]
    m = np.arange(2048)[None, :]
    blk = p // 32
    i = p % 32
    q = m >> 8
    r = (m >> 6) & 3
    b = m & 63
    n = 2048 * r + 32 * b + i
    det = 2 * CH * q + CH * blk + n
    return det.ravel()


def kernel(detection_depths, camera_depths, detection_times, camera_times):
    from concourse.bass_utils import run_bass_kernel_spmd

    if "nc" not in _CACHE:
        _CACHE["nc"] = _build_module()
        _CACHE["perm"] = _det_perm()
    nc = _CACHE["nc"]
    perm = _CACHE["perm"]

    dd = np.ascontiguousarray(np.asarray(detection_depths, np.float32))
    dt = np.ascontiguousarray(np.asarray(detection_times, np.float32))
    stat, jb, ones = _host_consts(camera_depths, camera_times)

    in_maps = []
    for c in range(N_CORES):
        sl = slice(c * N_C, (c + 1) * N_C)
        in_maps.append({
            "ddep": dd[sl].copy(),
            "dtim": dt[sl].copy(),
            "stat": stat,
            "jbias": jb,
            "ones4": ones,
        })
    results = run_bass_kernel_spmd(nc, in_maps, list(range(N_CORES))).results

    assignments = np.empty(N_TOTAL, np.int32)
    weights = np.empty(N_TOTAL, np.float32)
    for c in range(N_CORES):
        base = c * N_C
        a_loc = np.empty(N_C, np.int32)
        w_loc = np.empty(N_C, np.float32)
        a_loc[perm] = results[c]["asn"].ravel()
        w_loc[perm] = results[c]["wts"].ravel()
        assignments[base:base + N_C] = a_loc
        weights[base:base + N_C] = w_loc
    return assignments, weights
